# revision 16
# baseline (speedup 1.0000x reference)
"""Trainium2 Bass kernel for BoxMultiHeadedAttention (B=8, N=512, D=512, H=8).

Sharding: data-parallel over batch — each of the 8 NeuronCores computes one
batch element end-to-end; weights replicated; no collectives.

Sparsity compaction (host-side, per call; sizes padded to the max over the
8 batch elements so a single SPMD program serves all cores):
  * keys with mask==0 contribute exp(-1e9)=0 -> dropped entirely;
    kept keys ordered [mask&obj ("geo" keys) | mask&~obj], padded to
    NRB5*128 with -1e9 mask columns.
  * queries permuted obj-first: the geometry bias only applies to
    (obj_i & obj_j) pairs, so wg is computed for geo-keys x obj-queries
    only; per-core residual regions are neutralized with data
    ([P,1] bias/clip vectors and an obj-query column mask).
  * output rows are inverse-permuted on the host.

Per-core algorithm (layout [keys(part), queries(free)] throughout):
  * x shipped bf16 as one concatenated [xq;xk;xv] matrix -> 4 XBAR
    DMA-transposes; all weights in one packed DMA; all f32/bf16 consts in
    one packed DMA each (DMA issue is latency-chained, so count matters).
  * geometry: g = clip(ln((dx/w_i)^2), C2) on DVE+ACT; phases t = a/(4pi)*g
    via f32 selector matmul; sin/cos by exact magic-number folds
    (sin(2pi t) = Sin(-2pi*(round(t)-t)); cos via round(t+1/4) and
    bias pi/2); per-head contraction on PE (bf16); dw/dh separable
    rank-64 bank contraction.
  * wg multiplier M = 1 + max(wg+bG-1, 1e-6-1)*objq masked per-core via
    [P,1] vectors; routed to attention layout through a DRAM roundtrip
    (plain-SBUF DMAs; the (h,g) permutation lives in DRAM-side APs).
  * exp-domain softmax: T = E * M on the geo sub-tile only; row sums via
    ones-matmul; 1/rowsum broadcast across partitions by an exact f32
    selection matmul (no DRAM roundtrip); output projection bias folded
    in as a ones-row matmul.
All loops are software-pipelined (finalization of iter g emitted after the
start of iter g+1) so the in-order engine queues never head-of-line block.
"""
import math
import numpy as np
import ml_dtypes
from contextlib import ExitStack

import concourse.bass as bass
import concourse.mybir as mybir
import concourse.tile as tile
from concourse.bass_utils import run_bass_kernel_spmd

F32 = mybir.dt.float32
BF16 = mybir.dt.bfloat16
AF = mybir.ActivationFunctionType
ALU = mybir.AluOpType

B, N, D, H = 8, 512, 512, 8
DK = D // H
P = 128
NRB = N // P
GM = 16
WAVE_LEN = 1000.0
MAGIC = 12582912.0
C2 = float(2.0 * math.log(0.001))
ESHIFT = -6.0
TWO_PI = float(2.0 * math.pi)
HALF_PI = float(math.pi / 2.0)

_alphas = (100.0 / (WAVE_LEN ** (np.arange(8) / 8.0))).astype(np.float64)
BF = ml_dtypes.bfloat16


def _split_multi_waits(nc):
    """walrus accepts only ONE sync-wait per ISA instruction; hoist extras
    onto NoOps inserted before the offending instruction."""
    n_fix = 0
    for blk in nc.main_func.blocks:
        insts = list(blk.instructions)
        out, dirty = [], False
        for inst in insts:
            si = inst.sync_info
            waits = list(si.on_wait) if si is not None else []
            if len(waits) > 1:
                for kk, w in enumerate(waits[:-1]):
                    out.append(mybir.InstNoOp(
                        name=f"I-waitfix-{n_fix}-{kk}", engine=inst.engine,
                        sync_info=mybir.SyncInfo(on_wait=[w], on_update=[])))
                inst.sync_info = mybir.SyncInfo(
                    on_wait=[waits[-1]], on_update=list(si.on_update))
                n_fix += 1
                dirty = True
            out.append(inst)
        if dirty:
            blk.instructions = out
    return n_fix


def _selector_const():
    # SELAP[64*W + q*16 + m_loc, q*128 + m_loc*8 + j] = alpha_j/(4pi)
    selap = np.zeros((P, 4, P), dtype=np.float32)
    for W in range(2):
        for q in range(4):
            for m_loc in range(GM):
                for j in range(8):
                    selap[64 * W + q * 16 + m_loc, q, m_loc * 8 + j] = \
                        _alphas[j] / (4.0 * math.pi)
    return selap.reshape(P, 4 * P)


def _onehot8():
    oh = np.zeros((P, H, H), dtype=np.float32)
    for h in range(H):
        oh[:, h, h] = 1.0
    return oh.reshape(P, H * H)


def _wblk_direct(WG):
    # direct sin/cos weights: c in (sin-x, cos-x, sin-y, cos-y)
    gmap = [lambda j: j, lambda j: 32 + j, lambda j: 8 + j, lambda j: 40 + j]
    wblk = np.zeros((P, 4, P), dtype=np.float32)
    for c in range(4):
        for m_loc in range(GM):
            for j in range(8):
                for h in range(H):
                    wblk[m_loc * 8 + j, c, h * GM + m_loc] = WG[h, gmap[c](j)]
    return wblk.reshape(P, 4 * P)


def _bank_consts(WG):
    # dw/dh rank-64 decomposition (sin(A-B) via quarter-phase shifts)
    acol = np.zeros((64, 1), np.float32)
    pcol_m = np.zeros((64, 1), np.float32)
    pcol_n = np.zeros((64, 1), np.float32)
    w1 = np.zeros((64, H), np.float32)
    for f in range(2):
        for j in range(8):
            gs = 16 + 8 * f + j
            gc = 48 + 8 * f + j
            a = _alphas[j] / (4.0 * math.pi)
            for t in range(4):
                k = (f * 8 + j) * 4 + t
                acol[k, 0] = a
                pcol_m[k, 0] = 0.25 if t in (0, 2) else 0.0
                if t == 0:
                    pcol_n[k, 0] = 0.0; w1[k] = WG[:, gs]
                elif t == 1:
                    pcol_n[k, 0] = 0.75; w1[k] = WG[:, gs]   # -cos -> +pi
                elif t == 2:
                    pcol_n[k, 0] = 0.25; w1[k] = WG[:, gc]
                else:
                    pcol_n[k, 0] = 0.0; w1[k] = WG[:, gc]
    w1e = np.repeat(w1, GM, axis=1).astype(np.float32)
    return acol, pcol_m, pcol_n, w1e


def _bc8_const():
    # rr_b[p, n] = rs2[p//64, n]: bc8[k, p] = 1 iff k == p//64
    bc8 = np.zeros((2, P), np.float32)
    for p_ in range(P):
        bc8[p_ // 64, p_] = 1.0
    return bc8


def _host_prep(inputs):
    q = np.asarray(inputs["input_query"], np.float32)
    k = np.asarray(inputs["input_key"], np.float32)
    v = np.asarray(inputs["input_value"], np.float32)
    box = np.asarray(inputs["input_box"], np.float32)
    mask = np.asarray(inputs["mask"])
    nobj = np.asarray(inputs["not_objects"])
    WG = np.asarray(inputs["WG"], np.float32)
    bG = np.asarray(inputs["bG"], np.float32)

    x_min, y_min, x_max, y_max = [box[..., i] for i in range(4)]
    cx = (x_min + x_max) * 0.5
    cy = (y_min + y_max) * 0.5
    ww = x_max - x_min + 1.0
    hh = y_max - y_min + 1.0
    l2w = (2.0 * np.log(ww)).astype(np.float32)
    l2h = (2.0 * np.log(hh)).astype(np.float32)

    keyo, qo, G5s, K5s, Q5s = [], [], [], [], []
    for b in range(B):
        m_b = mask[b] != 0
        o_b = ~nobj[b]
        geo = np.where(m_b & o_b)[0]
        oth = np.where(m_b & ~o_b)[0]
        keyo.append(np.concatenate([geo, oth]))
        qobj = np.where(o_b)[0]
        qrest = np.where(~o_b)[0]
        qo.append(np.concatenate([qobj, qrest]))
        G5s.append(len(geo)); K5s.append(len(geo) + len(oth))
        Q5s.append(len(qobj))

    G5max = max(max(G5s), 1)
    n_geo = (G5max + GM - 1) // GM
    G5pad = n_geo * GM
    GBLK = (G5pad + P - 1) // P
    K5max = max(max(K5s), 1)
    NRB5 = (K5max + P - 1) // P
    K5pad = NRB5 * P
    Q5max = max(max(Q5s), 1)
    Q5pad = min(N, ((Q5max + 31) // 32) * 32)
    sizes = (n_geo, GBLK, NRB5, Q5pad)
    CW = 4 + 4 + NRB5 + GBLK + GBLK + n_geo + n_geo

    acol, pcol_m, pcol_n, w1e = _bank_consts(WG)
    # shared bf16 pack: oh8(64) | wblk(512) | bvbo(512)
    b16p = np.zeros((P, 64 + 512 + 512), np.float32)
    b16p[:, 0:64] = _onehot8()
    b16p[:, 64:576] = _wblk_direct(WG)
    b16p[0, 576:1088] = np.asarray(inputs["bv"], np.float32)
    b16p[32, 576:1088] = np.asarray(inputs["bo"], np.float32)
    wall = np.concatenate([
        np.asarray(inputs["Wq"], np.float32),
        np.asarray(inputs["Wk"], np.float32),
        np.asarray(inputs["Wv"], np.float32),
        np.asarray(inputs["Wo"], np.float32)], axis=1)  # [512, 2048]
    shared = {
        "wall": wall.astype(BF),
        "b16p": b16p.astype(BF),
    }
    bqc = np.asarray(inputs["bq"], np.float32).reshape(NRB, P).T
    bk8c = np.asarray(inputs["bk"], np.float32).reshape(NRB, P).T

    # f32 pack layout: selap(512) | cpack(CW) | w1e(128) | cp64(3) | bc8(512)
    F32W = 512 + CW + 128 + 3 + 128
    f32_base = np.zeros((P, F32W), np.float32)
    f32_base[:, 0:512] = _selector_const()
    f32_base[0:64, 512 + CW:512 + CW + 128] = w1e
    f32_base[0:64, 512 + CW + 128:512 + CW + 131] = \
        np.concatenate([acol, pcol_m, pcol_n], axis=1)
    f32_base[0:2, 512 + CW + 131:512 + CW + 259] = _bc8_const()

    in_maps = []
    for b in range(B):
        ko, qp = keyo[b], qo[b]
        G5, K5, Q5 = G5s[b], K5s[b], Q5s[b]

        xall = np.zeros((N + 2 * K5pad, D), BF)
        xall[0:N] = q[b][qp].astype(BF)
        xall[N:N + K5] = k[b][ko].astype(BF)
        xall[N + K5pad:N + K5pad + K5] = v[b][ko].astype(BF)

        cxk = np.zeros(GBLK * P, np.float32); cxk[:G5] = cx[b][ko[:G5]]
        cyk = np.zeros(GBLK * P, np.float32); cyk[:G5] = cy[b][ko[:G5]]
        l2wk = np.zeros(G5pad, np.float32); l2wk[:G5] = l2w[b][ko[:G5]]
        l2hk = np.zeros(G5pad, np.float32); l2hk[:G5] = l2h[b][ko[:G5]]
        bq4 = np.zeros((4, Q5pad), np.float32)
        bq4[2:] = 1.0
        l2q2 = np.zeros((2, Q5pad), np.float32)
        nq = min(Q5pad, N)
        bq4[0, :nq] = cx[b][qp[:nq]]; bq4[1, :nq] = cy[b][qp[:nq]]
        bq4[2, :nq] = 1.0 / ww[b][qp[:nq]]; bq4[3, :nq] = 1.0 / hh[b][qp[:nq]]
        l2q2[0, :nq] = l2w[b][qp[:nq]]; l2q2[1, :nq] = l2h[b][qp[:nq]]
        objq = np.zeros(Q5pad, np.float32)
        objq[:min(Q5, Q5pad)] = 1.0

        maskcol = np.full(NRB5 * P, -1e9 + ESHIFT, np.float32)
        maskcol[:K5] = ESHIFT
        maskcol = maskcol.reshape(NRB5, P).T

        bgm1 = np.zeros((P, n_geo), np.float32)
        epsm1 = np.zeros((P, n_geo), np.float32)
        for g in range(n_geo):
            for m in range(GM):
                key = g * GM + m
                for h in range(H):
                    if key < G5:
                        bgm1[h * GM + m, g] = bG[h] - 1.0
                        epsm1[h * GM + m, g] = 1e-6 - 1.0
                    else:
                        bgm1[h * GM + m, g] = -1e9
                        epsm1[h * GM + m, g] = 0.0

        f32p = f32_base.copy()
        f32p[:, 512:512 + CW] = np.concatenate([
            bqc, bk8c, maskcol,
            cxk.reshape(GBLK, P).T, cyk.reshape(GBLK, P).T,
            bgm1, epsm1], axis=1)

        l2kM = np.concatenate([np.broadcast_to(l2wk, (32, G5pad)),
                               np.broadcast_to(l2hk, (32, G5pad))], axis=0)
        l2qNh = np.concatenate([np.broadcast_to(l2q2[0], (32, Q5pad)),
                                np.broadcast_to(l2q2[1], (32, Q5pad))], axis=0)

        mm = dict(shared)
        mm.update({
            "xall": xall,
            "f32p": np.ascontiguousarray(f32p),
            "bq4": bq4,
            "l2kM": np.ascontiguousarray(l2kM),
            "l2qN": np.ascontiguousarray(l2qNh),
            "objq": objq.astype(BF),
        })
        in_maps.append(mm)

    inv_q = [np.argsort(qp) for qp in qo]
    return in_maps, sizes, inv_q


def build_nc(n_geo, GBLK, NRB5, Q5pad):
    K5pad = NRB5 * P
    G5pad = n_geo * GM
    XR = N + 2 * K5pad
    CW = 4 + 4 + NRB5 + GBLK + GBLK + n_geo + n_geo
    F32W = 512 + CW + 128 + 3 + 128
    nc = bass.Bass()

    def dp(name, shape, dt=F32):
        return nc.declare_dram_parameter(name, list(shape), dt, isOutput=False)

    XALL = dp("xall", (XR, D), BF16)
    WALL = dp("wall", (D, 4 * D), BF16)
    F32P = dp("f32p", (P, F32W))
    B16P = dp("b16p", (P, 1088), BF16)
    BQ4 = dp("bq4", (4, Q5pad))
    L2KM = dp("l2kM", (64, G5pad))
    L2QN = dp("l2qN", (64, Q5pad))
    OBJQ = dp("objq", (Q5pad,), BF16)
    out = nc.declare_dram_parameter("out", [N, D], F32, isOutput=True)
    wgd_dram = nc.dram_tensor("wgd_scratch", [n_geo, GM, H, Q5pad], BF16)

    with ExitStack() as ctx:
        tc = ctx.enter_context(tile.TileContext(nc))
        const = ctx.enter_context(tc.tile_pool(name="const", bufs=1))
        persist = ctx.enter_context(tc.tile_pool(name="persist", bufs=1))

        # ------------- constants (ACT queue) --------------------------------
        f32p = const.tile([P, F32W], F32, tag="f32p")
        nc.scalar.dma_start(f32p[:], F32P[:])
        CPo = 512
        W1o = 512 + CW
        C64o = W1o + 128
        BC8o = C64o + 3
        bq_t = f32p[:, CPo:CPo + 4]
        bk8x_t = f32p[:, CPo + 4:CPo + 8]
        mcol_t = f32p[:, CPo + 8:CPo + 8 + NRB5]
        cxk_t = f32p[:, CPo + 8 + NRB5:CPo + 8 + NRB5 + GBLK]
        cyk_t = f32p[:, CPo + 8 + NRB5 + GBLK:CPo + 8 + NRB5 + 2 * GBLK]
        bg_o = CPo + 8 + NRB5 + 2 * GBLK
        bgm1_t = f32p[:, bg_o:bg_o + n_geo]
        epsm1_t = f32p[:, bg_o + n_geo:bg_o + 2 * n_geo]
        w1e_f = f32p[0:64, W1o:W1o + 128]
        acol_t = f32p[0:64, C64o:C64o + 1]
        pcolm_t = f32p[0:64, C64o + 1:C64o + 2]
        pcoln_t = f32p[0:64, C64o + 2:C64o + 3]

        def selap(r0, r1, q4):
            return f32p[r0:r1, q4 * P:(q4 + 1) * P]

        bc8_t = f32p[0:2, BC8o:BC8o + P]

        b16p = const.tile([P, 1088], BF16, tag="b16p")
        nc.scalar.dma_start(b16p[:], B16P[:])

        def oh8(h):
            return b16p[:, h * 8:(h + 1) * 8]

        def wblk(c):
            return b16p[:, 64 + c * P:64 + (c + 1) * P]
        bvrow = b16p[0:1, 576:1088]
        borow = b16p[32:33, 576:1088]

        bq4bc = const.tile([P, 4, Q5pad], F32, tag="bq4bc")
        nc.scalar.dma_start(bq4bc[:],
                            BQ4[None, :, :].to_broadcast((P, 4, Q5pad)))
        cxqbc = bq4bc[:, 0, :]; cyqbc = bq4bc[:, 1, :]
        iwqbc = bq4bc[:, 2, :]; ihqbc = bq4bc[:, 3, :]
        objqbc = const.tile([P, Q5pad], BF16, tag="objqbc")
        nc.scalar.dma_start(objqbc[:], OBJQ[None, :].to_broadcast((P, Q5pad)))
        l2kM = const.tile([64, G5pad], F32, tag="l2kM")
        nc.scalar.dma_start(l2kM[:], L2KM[:])
        l2qN = const.tile([64, Q5pad], F32, tag="l2qN")
        nc.scalar.dma_start(l2qN[:], L2QN[:])
        halfpi_t = const.tile([P, 1], F32, tag="halfpi")
        nc.vector.memset(halfpi_t[:], HALF_PI)
        oh2 = const.tile([P, 4], BF16, tag="oh2")
        nc.vector.memset(oh2[:], 0.0)
        nc.vector.memset(oh2[:, 0:1], 1.0)
        nc.vector.memset(oh2[:, 3:4], 1.0)
        ones33_bf = const.tile([33, P], BF16, tag="ones33")
        nc.vector.memset(ones33_bf[:], 1.0)

        # ------------- input loads first (PE critical path, SP queue) -------
        xallT = persist.tile([P, NRB, XR], BF16, tag="xallT")
        wall = persist.tile([P, NRB, 4 * D], BF16, tag="wall")
        for cb in range(NRB):
            nc.sync.dma_start_transpose(xallT[:, cb, :],
                                        XALL[:, cb * P:(cb + 1) * P])
        nc.sync.dma_start(wall[:], WALL.rearrange("(kb p) d -> p kb d", p=P))

        def xqT(kb):
            return xallT[:, kb, 0:N]

        def xkT(kb):
            return xallT[:, kb, N:N + K5pad]

        def xvT(kb, c0, c1):
            return xallT[:, kb, N + K5pad + c0:N + K5pad + c1]

        def wsl(wi, kb, c0, c1):
            return wall[:, kb, wi * D + c0:wi * D + c1]

        # ---------------- phase 2: ln fields (pipelined) --------------------
        qT = persist.tile([P, NRB, N], BF16, tag="qT")
        kTt = persist.tile([P, NRB, K5pad], BF16, tag="kT")
        v_sb = persist.tile([P, NRB5, D], BF16, tag="v_sb")
        dxy2 = persist.tile([P, GBLK, 2, Q5pad], F32, tag="dxy2")
        with tc.tile_pool(name="work2", bufs=5) as work2:
            items = [(blk, ci) for blk in range(GBLK) for ci in range(2)]
            dws = {}
            l2s = {}
            for (blk, ci) in items:
                cbc = cxqbc if ci == 0 else cyqbc
                ccol = cxk_t if ci == 0 else cyk_t
                ibc = iwqbc if ci == 0 else ihqbc
                d_ = work2.tile([P, Q5pad], F32, tag="geo_d")
                nc.vector.tensor_scalar(d_[:], cbc, ccol[:, blk:blk + 1],
                                        None, ALU.subtract)
                dw_ = work2.tile([P, Q5pad], F32, tag="geo_dw")
                nc.vector.tensor_tensor(dw_[:], d_[:], ibc, ALU.mult)
                dws[(blk, ci)] = dw_
            for (blk, ci) in items:
                d2 = work2.tile([P, Q5pad], F32, tag="geo_d2")
                nc.scalar.activation(d2[:], dws[(blk, ci)][:], AF.Square)
                l2t = work2.tile([P, Q5pad], F32, tag="geo_l2")
                nc.scalar.activation(l2t[:], d2[:], AF.Ln)
                l2s[(blk, ci)] = l2t
            for (blk, ci) in items:
                nc.vector.tensor_scalar_max(dxy2[:, blk, ci, :],
                                            l2s[(blk, ci)][:], C2)

        # ---------------- phase 3: dw/dh banks ----------------
        bankM = persist.tile([64, G5pad], BF16, tag="bankM")
        bankN = persist.tile([64, Q5pad], BF16, tag="bankN")
        with tc.tile_pool(name="work3", bufs=2) as work3:
            bk_items = ((pcolm_t, l2kM, G5pad, bankM),
                        (pcoln_t, l2qN, Q5pad, bankN))
            fs = []
            for (pcol, l2bc, width, bank) in bk_items:
                t_ = work3.tile([64, width], F32, tag="bk_t")
                nc.vector.tensor_scalar(t_[:], l2bc[:], acol_t, pcol,
                                        ALU.mult, ALU.add)
                r_ = work3.tile([64, width], F32, tag="bk_r")
                nc.vector.tensor_scalar(r_[:], t_[:], MAGIC, -MAGIC,
                                        ALU.add, ALU.add)
                f_ = work3.tile([64, width], F32, tag="bk_f")
                nc.vector.tensor_tensor(f_[:], t_[:], r_[:], ALU.subtract)
                fs.append(f_)
            for (f_, (pcol, l2bc, width, bank)) in zip(fs, bk_items):
                nc.scalar.activation(bank[:], f_[:], AF.Sin, scale=TWO_PI)

        # ------- phase 4: geometry weights + interleaved projections --------
        # proj groups: (kind, idx); evictions on ACT so DVE stays on folds
        groups = ([("q", ob) for ob in range(NRB)]
                  + [("k", ob) for ob in range(NRB)]
                  + [("v", mb) for mb in range(NRB5)])
        NGRP = len(groups)

        wgdT = persist.tile([P, GBLK, H, Q5pad], BF16, tag="wgdT")
        with tc.tile_pool(name="work4", bufs=3) as work4, \
             tc.tile_pool(name="psum_u", bufs=2, space="PSUM") as psum_u, \
             tc.tile_pool(name="psum_p", bufs=2, space="PSUM") as psum_p, \
             tc.tile_pool(name="psum_wg", bufs=2, space="PSUM") as psum_wg:
            wgps = [None] * n_geo
            gps = [None] * NGRP

            def emit_group(j):
                kind, ob = groups[j]
                ps = psum_p.tile([P, N], F32, tag="pps")
                if kind == "q":
                    for kb in range(NRB):
                        nc.tensor.matmul(ps[:],
                                         wsl(0, kb, ob * P, (ob + 1) * P),
                                         xqT(kb),
                                         start=(kb == 0),
                                         stop=(kb == NRB - 1))
                elif kind == "k":
                    for kb in range(NRB):
                        nc.tensor.matmul(ps[:, :K5pad],
                                         wsl(1, kb, ob * P, (ob + 1) * P),
                                         xkT(kb),
                                         start=(kb == 0),
                                         stop=(kb == NRB - 1))
                else:
                    for kb in range(NRB):
                        nc.tensor.matmul(ps[:], xvT(kb, ob * P, (ob + 1) * P),
                                         wsl(2, kb, 0, D),
                                         start=(kb == 0), stop=False)
                    nc.tensor.matmul(ps[:], ones33_bf[0:1, :], bvrow,
                                     start=False, stop=True)
                gps[j] = ps

            def evict_group(j):
                kind, ob = groups[j]
                ps = gps[j]
                if kind == "q":
                    nc.scalar.activation(qT[:, ob, :], ps[:], AF.Identity,
                                         bias=bq_t[:, ob:ob + 1])
                elif kind == "k":
                    nc.scalar.activation(kTt[:, ob, :], ps[:, :K5pad],
                                         AF.Identity, scale=0.125,
                                         bias=bk8x_t[:, ob:ob + 1])
                else:
                    nc.scalar.copy(v_sb[:, ob, :], ps[:])

            # distribute groups over geo iters (emit_group(j) at iter sched[j])
            sched = {}
            g0 = max(1, n_geo // 3)
            span = max(1, n_geo - g0)
            for j in range(NGRP):
                sched.setdefault(min(g0 + j * span // NGRP, n_geo - 1),
                                 []).append(j)

            def stage_b(g):
                wgdB = work4.tile([P, Q5pad], BF16, tag="wgdB")
                nc.vector.tensor_scalar(wgdB[:], wgps[g][:, :Q5pad],
                                        bgm1_t[:, g:g + 1],
                                        epsm1_t[:, g:g + 1],
                                        ALU.add, ALU.max)
                wgdm1 = work4.tile([P, Q5pad], BF16, tag="wgdm1")
                nc.gpsimd.tensor_tensor(wgdm1[:], wgdB[:], objqbc[:],
                                        ALU.mult)
                wgdM = work4.tile([P, Q5pad], BF16, tag="wgdM")
                nc.gpsimd.tensor_scalar(wgdM[:], wgdm1[:], 1.0, None, ALU.add)
                nc.scalar.dma_start(
                    wgd_dram[g].rearrange("t h q -> h t q"), wgdM[:])

            prev_groups = []
            for g in range(n_geo):
                blk = g // 8
                off = 64 * ((g % 8) // 4)
                q4 = g % 4
                mbase = g * GM
                lhs_wh = work4.tile([64, P], BF16, tag="lhs_wh")
                nc.vector.tensor_tensor(
                    lhs_wh[:].rearrange("k (h m) -> k h m", h=H),
                    w1e_f.rearrange("k (h m) -> k h m", h=H),
                    bankM[:, mbase:mbase + GM][:, None, :]
                        .to_broadcast((64, H, GM)),
                    ALU.mult)
                ups = psum_u.tile([P, 2, N], F32, tag="ups")
                for ci in range(2):
                    nc.tensor.matmul(ups[:, ci, :Q5pad],
                                     selap(off, off + 64, q4),
                                     dxy2[off:off + 64, blk, ci, :],
                                     start=True, stop=True)
                # projections fill the PE gap while DVE folds
                for j in sched.get(g, []):
                    emit_group(j)
                upsv = ups[:, :, :Q5pad]
                rrS = work4.tile([P, 2, Q5pad], F32, tag="rrS")
                nc.vector.tensor_scalar(rrS[:], upsv, MAGIC, -MAGIC,
                                        ALU.add, ALU.add)
                nfS = work4.tile([P, 2, Q5pad], F32, tag="nfS")
                nc.vector.tensor_tensor(nfS[:], rrS[:], upsv, ALU.subtract)
                # cos fold from nfS: nfC = nfS + (nfS <= -0.25)
                ind = work4.tile([P, 2, Q5pad], F32, tag="ind")
                nc.gpsimd.tensor_scalar(ind[:], nfS[:], -0.25, None,
                                        ALU.is_le)
                nfC = work4.tile([P, 2, Q5pad], F32, tag="nfC")
                nc.vector.tensor_tensor(nfC[:], nfS[:], ind[:], ALU.add)
                sS = work4.tile([P, 2, Q5pad], BF16, tag="sS")
                nc.scalar.activation(sS[:], nfS[:], AF.Sin, scale=-TWO_PI)
                sC = work4.tile([P, 2, Q5pad], BF16, tag="sC")
                nc.scalar.activation(sC[:], nfC[:], AF.Sin, scale=-TWO_PI,
                                     bias=halfpi_t[:])
                wgp = psum_wg.tile([P, N], F32, tag="wgp")
                nc.tensor.matmul(wgp[:, :Q5pad], wblk(0), sS[:, 0, :],
                                 start=True, stop=False)
                nc.tensor.matmul(wgp[:, :Q5pad], wblk(1), sC[:, 0, :],
                                 start=False, stop=False)
                nc.tensor.matmul(wgp[:, :Q5pad], wblk(2), sS[:, 1, :],
                                 start=False, stop=False)
                nc.tensor.matmul(wgp[:, :Q5pad], wblk(3), sC[:, 1, :],
                                 start=False, stop=False)
                nc.tensor.matmul(wgp[:, :Q5pad], lhs_wh[:], bankN[:],
                                 start=False, stop=True)
                wgps[g] = wgp
                if g >= 1:
                    stage_b(g - 1)
                for j in prev_groups:
                    evict_group(j)
                prev_groups = sched.get(g, [])
            stage_b(n_geo - 1)
            for j in prev_groups:
                evict_group(j)
            # gather to attention layout: one DMA per block
            for blk in range(GBLK):
                gcnt = min(8, n_geo - blk * 8)
                nc.scalar.dma_start(
                    wgdT[0:gcnt * GM, blk, :, :],
                    wgd_dram[blk * 8:blk * 8 + gcnt]
                        .rearrange("g t h q -> (g t) h q"))

        # ---------------- phase 5: attention (2-stage pipeline) -------------
        ot = persist.tile([P, NRB, N], BF16, tag="ot")
        with tc.tile_pool(name="work5", bufs=4) as work5, \
             tc.tile_pool(name="psum5", bufs=2, space="PSUM") as psum5, \
             tc.tile_pool(name="psum_s", bufs=1, space="PSUM") as psum_s, \
             tc.tile_pool(name="psum_r", bufs=1, space="PSUM") as psum_r, \
             tc.tile_pool(name="psum_av", bufs=1, space="PSUM") as psum_av:
            flat = [(ob, rb) for ob in range(NRB) for rb in range(NRB5)]
            avs, sbanks, es = {}, {}, {}

            def accum(ob, rb):
                h0 = 2 * ob
                if rb == 0:
                    av_t = psum_av.tile([P, N], F32, tag="avps")
                    sb_t = psum_s.tile([2, N], F32, tag="sbank")
                    avs[ob] = av_t
                    sbanks[ob] = sb_t
                e_ = es[(ob, rb)]
                for hi in range(2):
                    po = hi * DK
                    nc.tensor.matmul(sbanks[ob][:], oh2[:, 2 * hi:2 * hi + 2],
                                     e_[:, hi, :],
                                     start=(rb == 0 and hi == 0),
                                     stop=(rb == NRB5 - 1 and hi == 1),
                                     skip_group_check=True)
                    nc.tensor.matmul(avs[ob][po:po + DK, :],
                                     v_sb[:, rb,
                                          (h0 + hi) * DK:(h0 + hi + 1) * DK],
                                     e_[:, hi, :], start=(rb == 0),
                                     stop=(rb == NRB5 - 1),
                                     skip_group_check=True)

            def finish(ob):
                rs = work5.tile([2, N], F32, tag="rs")
                nc.vector.reciprocal(rs[:], sbanks[ob][:])
                rrp = psum_r.tile([P, N], F32, tag="rrp")
                nc.tensor.matmul(rrp[:], bc8_t, rs[:], start=True, stop=True)
                rr_b = work5.tile([P, N], F32, tag="rr_b")
                nc.scalar.copy(rr_b[:], rrp[:])
                nc.vector.tensor_tensor(ot[:, ob, :], avs[ob][:], rr_b[:],
                                        ALU.mult)

            for i, (ob, rb) in enumerate(flat):
                h0 = 2 * ob
                st2 = psum5.tile([P, 2, N], F32, tag="stps")
                for hi in range(2):
                    po = hi * DK
                    nc.tensor.matmul(
                        st2[:, hi, :],
                        kTt[po:po + DK, ob, rb * P:(rb + 1) * P],
                        qT[po:po + DK, ob, :], start=True, stop=True)
                e_ = work5.tile([P, 2, N], BF16, tag="e_t")
                nc.scalar.activation(e_[:], st2[:], AF.Exp,
                                     bias=mcol_t[:, rb:rb + 1])
                if rb < GBLK:
                    rows = min(P, G5pad - rb * P)
                    nc.vector.tensor_tensor(
                        e_[0:rows, :, 0:Q5pad], e_[0:rows, :, 0:Q5pad],
                        wgdT[0:rows, rb, h0:h0 + 2, :], ALU.mult)
                es[(ob, rb)] = e_
                if i >= 2:
                    accum(*flat[i - 2])
                    if flat[i - 2][1] == NRB5 - 1:
                        finish(flat[i - 2][0])
            for i in (len(flat) - 2, len(flat) - 1):
                accum(*flat[i])
                if flat[i][1] == NRB5 - 1:
                    finish(flat[i][0])

        # ---------------- phase 6: output projection ----------------
        with tc.tile_pool(name="work6", bufs=2) as work6, \
             tc.tile_pool(name="psum6", bufs=2, space="PSUM") as psum6:
            fps = []

            def evict(rr):
                fo = work6.tile([P, D], F32, tag="fo")
                if rr % 2 == 0:
                    nc.scalar.copy(fo[:], fps[rr][:])
                else:
                    nc.vector.tensor_copy(fo[:], fps[rr][:])
                nc.sync.dma_start(out[rr * P:(rr + 1) * P, :], fo[:])

            for r in range(NRB):
                ps = psum6.tile([P, D], F32, tag="fps")
                for kt in range(NRB):
                    nc.tensor.matmul(ps[:], ot[:, kt, r * P:(r + 1) * P],
                                     wsl(3, kt, 0, D),
                                     start=(kt == 0), stop=False)
                nc.tensor.matmul(ps[:], ones33_bf[32:33, :], borow,
                                 start=False, stop=True)
                fps.append(ps)
                if r >= 1:
                    evict(r - 1)
            evict(NRB - 1)

    _split_multi_waits(nc)
    return nc


_NC_CACHE = {}


def kernel(**inputs):
    in_maps, sizes, inv_q = _host_prep(inputs)
    if _NC_CACHE.get("sizes") != sizes:
        _NC_CACHE["nc"] = build_nc(*sizes)
        _NC_CACHE["sizes"] = sizes
    nc = _NC_CACHE["nc"]
    res = run_bass_kernel_spmd(nc, in_maps, list(range(B)))
    out = np.stack([res.results[b]["out"][inv_q[b]] for b in range(B)], axis=0)
    return out.astype(np.float32)


if __name__ == "__main__":
    print("kernel module ok")


# revision 17
# speedup vs baseline: 1.0036x; 1.0036x over previous
"""Trainium2 Bass kernel for BoxMultiHeadedAttention (B=8, N=512, D=512, H=8).

Sharding: data-parallel over batch — each of the 8 NeuronCores computes one
batch element end-to-end; weights replicated; no collectives.

Sparsity compaction (host-side, per call; sizes padded to the max over the
8 batch elements so a single SPMD program serves all cores):
  * keys with mask==0 contribute exp(-1e9)=0 -> dropped entirely;
    kept keys ordered [mask&obj ("geo" keys) | mask&~obj], padded to
    NRB5*128 with -1e9 mask columns.
  * queries permuted obj-first: the geometry bias only applies to
    (obj_i & obj_j) pairs, so wg is computed for geo-keys x obj-queries
    only; per-core residual regions are neutralized with data
    ([P,1] bias/clip vectors and an obj-query column mask).
  * output rows are inverse-permuted on the host.

Per-core algorithm (layout [keys(part), queries(free)] throughout):
  * x shipped bf16 as one concatenated [xq;xk;xv] matrix -> 4 XBAR
    DMA-transposes; all weights in one packed DMA; all f32/bf16 consts in
    one packed DMA each (DMA issue is latency-chained, so count matters).
  * geometry: g = clip(ln((dx/w_i)^2), C2) on DVE+ACT; phases t = a/(4pi)*g
    via f32 selector matmul; sin/cos by exact magic-number folds
    (sin(2pi t) = Sin(-2pi*(round(t)-t)); cos via round(t+1/4) and
    bias pi/2); per-head contraction on PE (bf16); dw/dh separable
    rank-64 bank contraction.
  * wg multiplier M = 1 + max(wg+bG-1, 1e-6-1)*objq masked per-core via
    [P,1] vectors; routed to attention layout through a DRAM roundtrip
    (plain-SBUF DMAs; the (h,g) permutation lives in DRAM-side APs).
  * exp-domain softmax: T = E * M on the geo sub-tile only; row sums via
    ones-matmul; 1/rowsum broadcast across partitions by an exact f32
    selection matmul (no DRAM roundtrip); output projection bias folded
    in as a ones-row matmul.
All loops are software-pipelined (finalization of iter g emitted after the
start of iter g+1) so the in-order engine queues never head-of-line block.
"""
import math
import numpy as np
import ml_dtypes
from contextlib import ExitStack

import concourse.bass as bass
import concourse.mybir as mybir
import concourse.tile as tile
from concourse.bass_utils import run_bass_kernel_spmd

F32 = mybir.dt.float32
BF16 = mybir.dt.bfloat16
AF = mybir.ActivationFunctionType
ALU = mybir.AluOpType

B, N, D, H = 8, 512, 512, 8
DK = D // H
P = 128
NRB = N // P
GM = 16
WAVE_LEN = 1000.0
MAGIC = 12582912.0
C2 = float(2.0 * math.log(0.001))
ESHIFT = -6.0
TWO_PI = float(2.0 * math.pi)
HALF_PI = float(math.pi / 2.0)

_alphas = (100.0 / (WAVE_LEN ** (np.arange(8) / 8.0))).astype(np.float64)
BF = ml_dtypes.bfloat16


def _split_multi_waits(nc):
    """walrus accepts only ONE sync-wait per ISA instruction; hoist extras
    onto NoOps inserted before the offending instruction."""
    n_fix = 0
    for blk in nc.main_func.blocks:
        insts = list(blk.instructions)
        out, dirty = [], False
        for inst in insts:
            si = inst.sync_info
            waits = list(si.on_wait) if si is not None else []
            if len(waits) > 1:
                for kk, w in enumerate(waits[:-1]):
                    out.append(mybir.InstNoOp(
                        name=f"I-waitfix-{n_fix}-{kk}", engine=inst.engine,
                        sync_info=mybir.SyncInfo(on_wait=[w], on_update=[])))
                inst.sync_info = mybir.SyncInfo(
                    on_wait=[waits[-1]], on_update=list(si.on_update))
                n_fix += 1
                dirty = True
            out.append(inst)
        if dirty:
            blk.instructions = out
    return n_fix


def _selector_const():
    # SELAP[64*W + q*16 + m_loc, q*128 + m_loc*8 + j] = alpha_j/(4pi)
    selap = np.zeros((P, 4, P), dtype=np.float32)
    for W in range(2):
        for q in range(4):
            for m_loc in range(GM):
                for j in range(8):
                    selap[64 * W + q * 16 + m_loc, q, m_loc * 8 + j] = \
                        _alphas[j] / (4.0 * math.pi)
    return selap.reshape(P, 4 * P)


def _onehot8():
    oh = np.zeros((P, H, H), dtype=np.float32)
    for h in range(H):
        oh[:, h, h] = 1.0
    return oh.reshape(P, H * H)


def _wblk_direct(WG):
    # direct sin/cos weights: c in (sin-x, cos-x, sin-y, cos-y)
    gmap = [lambda j: j, lambda j: 32 + j, lambda j: 8 + j, lambda j: 40 + j]
    wblk = np.zeros((P, 4, P), dtype=np.float32)
    for c in range(4):
        for m_loc in range(GM):
            for j in range(8):
                for h in range(H):
                    wblk[m_loc * 8 + j, c, h * GM + m_loc] = WG[h, gmap[c](j)]
    return wblk.reshape(P, 4 * P)


def _bank_consts(WG):
    # dw/dh rank-64 decomposition (sin(A-B) via quarter-phase shifts)
    acol = np.zeros((64, 1), np.float32)
    pcol_m = np.zeros((64, 1), np.float32)
    pcol_n = np.zeros((64, 1), np.float32)
    w1 = np.zeros((64, H), np.float32)
    for f in range(2):
        for j in range(8):
            gs = 16 + 8 * f + j
            gc = 48 + 8 * f + j
            a = _alphas[j] / (4.0 * math.pi)
            for t in range(4):
                k = (f * 8 + j) * 4 + t
                acol[k, 0] = a
                pcol_m[k, 0] = 0.25 if t in (0, 2) else 0.0
                if t == 0:
                    pcol_n[k, 0] = 0.0; w1[k] = WG[:, gs]
                elif t == 1:
                    pcol_n[k, 0] = 0.75; w1[k] = WG[:, gs]   # -cos -> +pi
                elif t == 2:
                    pcol_n[k, 0] = 0.25; w1[k] = WG[:, gc]
                else:
                    pcol_n[k, 0] = 0.0; w1[k] = WG[:, gc]
    w1e = np.repeat(w1, GM, axis=1).astype(np.float32)
    return acol, pcol_m, pcol_n, w1e


def _bc8_const():
    # rr_b[p, n] = rs2[p//64, n]: bc8[k, p] = 1 iff k == p//64
    bc8 = np.zeros((2, P), np.float32)
    for p_ in range(P):
        bc8[p_ // 64, p_] = 1.0
    return bc8


def _host_prep(inputs):
    q = np.asarray(inputs["input_query"], np.float32)
    k = np.asarray(inputs["input_key"], np.float32)
    v = np.asarray(inputs["input_value"], np.float32)
    box = np.asarray(inputs["input_box"], np.float32)
    mask = np.asarray(inputs["mask"])
    nobj = np.asarray(inputs["not_objects"])
    WG = np.asarray(inputs["WG"], np.float32)
    bG = np.asarray(inputs["bG"], np.float32)

    x_min, y_min, x_max, y_max = [box[..., i] for i in range(4)]
    cx = (x_min + x_max) * 0.5
    cy = (y_min + y_max) * 0.5
    ww = x_max - x_min + 1.0
    hh = y_max - y_min + 1.0
    l2w = (2.0 * np.log(ww)).astype(np.float32)
    l2h = (2.0 * np.log(hh)).astype(np.float32)

    keyo, qo, G5s, K5s, Q5s = [], [], [], [], []
    for b in range(B):
        m_b = mask[b] != 0
        o_b = ~nobj[b]
        geo = np.where(m_b & o_b)[0]
        oth = np.where(m_b & ~o_b)[0]
        keyo.append(np.concatenate([geo, oth]))
        qobj = np.where(o_b)[0]
        qrest = np.where(~o_b)[0]
        qo.append(np.concatenate([qobj, qrest]))
        G5s.append(len(geo)); K5s.append(len(geo) + len(oth))
        Q5s.append(len(qobj))

    G5max = max(max(G5s), 1)
    n_geo = (G5max + GM - 1) // GM
    G5pad = n_geo * GM
    GBLK = (G5pad + P - 1) // P
    K5max = max(max(K5s), 1)
    NRB5 = (K5max + P - 1) // P
    K5pad = NRB5 * P
    Q5max = max(max(Q5s), 1)
    Q5pad = min(N, ((Q5max + 31) // 32) * 32)
    sizes = (n_geo, GBLK, NRB5, Q5pad)
    CW = 4 + 4 + NRB5 + GBLK + GBLK + n_geo + n_geo

    acol, pcol_m, pcol_n, w1e = _bank_consts(WG)
    # shared bf16 pack: oh8(64) | wblk(512) | bvbo(512)
    b16p = np.zeros((P, 64 + 512 + 512), np.float32)
    b16p[:, 0:64] = _onehot8()
    b16p[:, 64:576] = _wblk_direct(WG)
    b16p[0, 576:1088] = np.asarray(inputs["bv"], np.float32)
    b16p[32, 576:1088] = np.asarray(inputs["bo"], np.float32)
    wall = np.concatenate([
        np.asarray(inputs["Wq"], np.float32),
        np.asarray(inputs["Wk"], np.float32),
        np.asarray(inputs["Wv"], np.float32),
        np.asarray(inputs["Wo"], np.float32)], axis=1)  # [512, 2048]
    shared = {
        "wall": wall.astype(BF),
        "b16p": b16p.astype(BF),
    }
    bqc = np.asarray(inputs["bq"], np.float32).reshape(NRB, P).T
    bk8c = np.asarray(inputs["bk"], np.float32).reshape(NRB, P).T

    # f32 pack layout: selap(512) | cpack(CW) | w1e(128) | cp64(3) | bc8(512)
    F32W = 512 + CW + 128 + 3 + 128
    f32_base = np.zeros((P, F32W), np.float32)
    f32_base[:, 0:512] = _selector_const()
    f32_base[0:64, 512 + CW:512 + CW + 128] = w1e
    f32_base[0:64, 512 + CW + 128:512 + CW + 131] = \
        np.concatenate([acol, pcol_m, pcol_n], axis=1)
    f32_base[0:2, 512 + CW + 131:512 + CW + 259] = _bc8_const()

    in_maps = []
    for b in range(B):
        ko, qp = keyo[b], qo[b]
        G5, K5, Q5 = G5s[b], K5s[b], Q5s[b]

        xall = np.zeros((N + 2 * K5pad, D), BF)
        xall[0:N] = q[b][qp].astype(BF)
        xall[N:N + K5] = k[b][ko].astype(BF)
        xall[N + K5pad:N + K5pad + K5] = v[b][ko].astype(BF)

        cxk = np.zeros(GBLK * P, np.float32); cxk[:G5] = cx[b][ko[:G5]]
        cyk = np.zeros(GBLK * P, np.float32); cyk[:G5] = cy[b][ko[:G5]]
        l2wk = np.zeros(G5pad, np.float32); l2wk[:G5] = l2w[b][ko[:G5]]
        l2hk = np.zeros(G5pad, np.float32); l2hk[:G5] = l2h[b][ko[:G5]]
        bq4 = np.zeros((4, Q5pad), np.float32)
        bq4[2:] = 1.0
        l2q2 = np.zeros((2, Q5pad), np.float32)
        nq = min(Q5pad, N)
        bq4[0, :nq] = cx[b][qp[:nq]]; bq4[1, :nq] = cy[b][qp[:nq]]
        bq4[2, :nq] = 1.0 / ww[b][qp[:nq]]; bq4[3, :nq] = 1.0 / hh[b][qp[:nq]]
        l2q2[0, :nq] = l2w[b][qp[:nq]]; l2q2[1, :nq] = l2h[b][qp[:nq]]
        objq = np.zeros(Q5pad, np.float32)
        objq[:min(Q5, Q5pad)] = 1.0

        maskcol = np.full(NRB5 * P, -1e9 + ESHIFT, np.float32)
        maskcol[:K5] = ESHIFT
        maskcol = maskcol.reshape(NRB5, P).T

        bgm1 = np.zeros((P, n_geo), np.float32)
        epsm1 = np.zeros((P, n_geo), np.float32)
        for g in range(n_geo):
            for m in range(GM):
                key = g * GM + m
                for h in range(H):
                    if key < G5:
                        bgm1[h * GM + m, g] = bG[h] - 1.0
                        epsm1[h * GM + m, g] = 1e-6 - 1.0
                    else:
                        bgm1[h * GM + m, g] = -1e9
                        epsm1[h * GM + m, g] = 0.0

        f32p = f32_base.copy()
        f32p[:, 512:512 + CW] = np.concatenate([
            bqc, bk8c, maskcol,
            cxk.reshape(GBLK, P).T, cyk.reshape(GBLK, P).T,
            bgm1, epsm1], axis=1)

        l2kM = np.concatenate([np.broadcast_to(l2wk, (32, G5pad)),
                               np.broadcast_to(l2hk, (32, G5pad))], axis=0)
        l2qNh = np.concatenate([np.broadcast_to(l2q2[0], (32, Q5pad)),
                                np.broadcast_to(l2q2[1], (32, Q5pad))], axis=0)

        mm = dict(shared)
        mm.update({
            "xall": xall,
            "f32p": np.ascontiguousarray(f32p),
            "bq4": bq4,
            "l2kM": np.ascontiguousarray(l2kM),
            "l2qN": np.ascontiguousarray(l2qNh),
            "objq": objq.astype(BF),
        })
        in_maps.append(mm)

    inv_q = [np.argsort(qp) for qp in qo]
    return in_maps, sizes, inv_q


def build_nc(n_geo, GBLK, NRB5, Q5pad):
    K5pad = NRB5 * P
    G5pad = n_geo * GM
    XR = N + 2 * K5pad
    CW = 4 + 4 + NRB5 + GBLK + GBLK + n_geo + n_geo
    F32W = 512 + CW + 128 + 3 + 128
    nc = bass.Bass()

    def dp(name, shape, dt=F32):
        return nc.declare_dram_parameter(name, list(shape), dt, isOutput=False)

    XALL = dp("xall", (XR, D), BF16)
    WALL = dp("wall", (D, 4 * D), BF16)
    F32P = dp("f32p", (P, F32W))
    B16P = dp("b16p", (P, 1088), BF16)
    BQ4 = dp("bq4", (4, Q5pad))
    L2KM = dp("l2kM", (64, G5pad))
    L2QN = dp("l2qN", (64, Q5pad))
    OBJQ = dp("objq", (Q5pad,), BF16)
    out = nc.declare_dram_parameter("out", [N, D], F32, isOutput=True)
    wgd_dram = nc.dram_tensor("wgd_scratch", [n_geo, GM, H, Q5pad], BF16)

    with ExitStack() as ctx:
        tc = ctx.enter_context(tile.TileContext(nc))
        const = ctx.enter_context(tc.tile_pool(name="const", bufs=1))
        persist = ctx.enter_context(tc.tile_pool(name="persist", bufs=1))

        # ------------- constants (ACT queue) --------------------------------
        f32p = const.tile([P, F32W], F32, tag="f32p")
        nc.scalar.dma_start(f32p[:], F32P[:])
        CPo = 512
        W1o = 512 + CW
        C64o = W1o + 128
        BC8o = C64o + 3
        bq_t = f32p[:, CPo:CPo + 4]
        bk8x_t = f32p[:, CPo + 4:CPo + 8]
        mcol_t = f32p[:, CPo + 8:CPo + 8 + NRB5]
        cxk_t = f32p[:, CPo + 8 + NRB5:CPo + 8 + NRB5 + GBLK]
        cyk_t = f32p[:, CPo + 8 + NRB5 + GBLK:CPo + 8 + NRB5 + 2 * GBLK]
        bg_o = CPo + 8 + NRB5 + 2 * GBLK
        bgm1_t = f32p[:, bg_o:bg_o + n_geo]
        epsm1_t = f32p[:, bg_o + n_geo:bg_o + 2 * n_geo]
        w1e_f = f32p[0:64, W1o:W1o + 128]
        acol_t = f32p[0:64, C64o:C64o + 1]
        pcolm_t = f32p[0:64, C64o + 1:C64o + 2]
        pcoln_t = f32p[0:64, C64o + 2:C64o + 3]

        def selap(r0, r1, q4):
            return f32p[r0:r1, q4 * P:(q4 + 1) * P]

        bc8_t = f32p[0:2, BC8o:BC8o + P]

        b16p = const.tile([P, 1088], BF16, tag="b16p")
        nc.scalar.dma_start(b16p[:], B16P[:])

        def oh8(h):
            return b16p[:, h * 8:(h + 1) * 8]

        def wblk(c):
            return b16p[:, 64 + c * P:64 + (c + 1) * P]
        bvrow = b16p[0:1, 576:1088]
        borow = b16p[32:33, 576:1088]

        bq4bc = const.tile([P, 4, Q5pad], F32, tag="bq4bc")
        nc.scalar.dma_start(bq4bc[:],
                            BQ4[None, :, :].to_broadcast((P, 4, Q5pad)))
        cxqbc = bq4bc[:, 0, :]; cyqbc = bq4bc[:, 1, :]
        iwqbc = bq4bc[:, 2, :]; ihqbc = bq4bc[:, 3, :]
        objqbc = const.tile([P, Q5pad], BF16, tag="objqbc")
        nc.scalar.dma_start(objqbc[:], OBJQ[None, :].to_broadcast((P, Q5pad)))
        l2kM = const.tile([64, G5pad], F32, tag="l2kM")
        nc.scalar.dma_start(l2kM[:], L2KM[:])
        l2qN = const.tile([64, Q5pad], F32, tag="l2qN")
        nc.scalar.dma_start(l2qN[:], L2QN[:])
        halfpi_t = const.tile([P, 1], F32, tag="halfpi")
        nc.vector.memset(halfpi_t[:], HALF_PI)
        oh2 = const.tile([P, 4], BF16, tag="oh2")
        nc.vector.memset(oh2[:], 0.0)
        nc.vector.memset(oh2[:, 0:1], 1.0)
        nc.vector.memset(oh2[:, 3:4], 1.0)
        ones33_bf = const.tile([33, P], BF16, tag="ones33")
        nc.vector.memset(ones33_bf[:], 1.0)

        # ------------- input loads first (PE critical path, SP queue) -------
        xallT = persist.tile([P, NRB, XR], BF16, tag="xallT")
        wall = persist.tile([P, NRB, 4 * D], BF16, tag="wall")
        for cb in range(NRB):
            nc.sync.dma_start_transpose(xallT[:, cb, :],
                                        XALL[:, cb * P:(cb + 1) * P])
        nc.sync.dma_start(wall[:], WALL.rearrange("(kb p) d -> p kb d", p=P))

        def xqT(kb):
            return xallT[:, kb, 0:N]

        def xkT(kb):
            return xallT[:, kb, N:N + K5pad]

        def xvT(kb, c0, c1):
            return xallT[:, kb, N + K5pad + c0:N + K5pad + c1]

        def wsl(wi, kb, c0, c1):
            return wall[:, kb, wi * D + c0:wi * D + c1]

        # ---------------- phase 2: ln fields (pipelined) --------------------
        qT = persist.tile([P, NRB, N], BF16, tag="qT")
        kTt = persist.tile([P, NRB, K5pad], BF16, tag="kT")
        v_sb = persist.tile([P, NRB5, D], BF16, tag="v_sb")
        dxy2 = persist.tile([P, GBLK, 2, Q5pad], F32, tag="dxy2")
        with tc.tile_pool(name="work2", bufs=5) as work2:
            items = [(blk, ci) for blk in range(GBLK) for ci in range(2)]
            dws = {}
            l2s = {}
            for (blk, ci) in items:
                cbc = cxqbc if ci == 0 else cyqbc
                ccol = cxk_t if ci == 0 else cyk_t
                ibc = iwqbc if ci == 0 else ihqbc
                d_ = work2.tile([P, Q5pad], F32, tag="geo_d")
                nc.vector.tensor_scalar(d_[:], cbc, ccol[:, blk:blk + 1],
                                        None, ALU.subtract)
                dw_ = work2.tile([P, Q5pad], F32, tag="geo_dw")
                nc.vector.tensor_tensor(dw_[:], d_[:], ibc, ALU.mult)
                dws[(blk, ci)] = dw_
            for (blk, ci) in items:
                d2 = work2.tile([P, Q5pad], F32, tag="geo_d2")
                nc.scalar.activation(d2[:], dws[(blk, ci)][:], AF.Square)
                l2t = work2.tile([P, Q5pad], F32, tag="geo_l2")
                nc.scalar.activation(l2t[:], d2[:], AF.Ln)
                l2s[(blk, ci)] = l2t
            for (blk, ci) in items:
                nc.vector.tensor_scalar_max(dxy2[:, blk, ci, :],
                                            l2s[(blk, ci)][:], C2)

        # ---------------- phase 3: dw/dh banks ----------------
        bankM = persist.tile([64, G5pad], BF16, tag="bankM")
        bankN = persist.tile([64, Q5pad], BF16, tag="bankN")
        with tc.tile_pool(name="work3", bufs=2) as work3:
            bk_items = ((pcolm_t, l2kM, G5pad, bankM),
                        (pcoln_t, l2qN, Q5pad, bankN))
            fs = []
            for (pcol, l2bc, width, bank) in bk_items:
                t_ = work3.tile([64, width], F32, tag="bk_t")
                nc.vector.tensor_scalar(t_[:], l2bc[:], acol_t, pcol,
                                        ALU.mult, ALU.add)
                r_ = work3.tile([64, width], F32, tag="bk_r")
                nc.vector.tensor_scalar(r_[:], t_[:], MAGIC, -MAGIC,
                                        ALU.add, ALU.add)
                f_ = work3.tile([64, width], F32, tag="bk_f")
                nc.vector.tensor_tensor(f_[:], t_[:], r_[:], ALU.subtract)
                fs.append(f_)
            for (f_, (pcol, l2bc, width, bank)) in zip(fs, bk_items):
                nc.scalar.activation(bank[:], f_[:], AF.Sin, scale=TWO_PI)

        # ------- phase 4: geometry weights + interleaved projections --------
        # proj groups: (kind, idx); evictions on ACT so DVE stays on folds
        groups = ([("q", ob) for ob in range(NRB)]
                  + [("k", ob) for ob in range(NRB)]
                  + [("v", mb) for mb in range(NRB5)])
        NGRP = len(groups)

        wgdT = persist.tile([P, GBLK, H, Q5pad], BF16, tag="wgdT")
        with tc.tile_pool(name="work4", bufs=3) as work4, \
             tc.tile_pool(name="psum_u", bufs=2, space="PSUM") as psum_u, \
             tc.tile_pool(name="psum_p", bufs=2, space="PSUM") as psum_p, \
             tc.tile_pool(name="psum_wg", bufs=2, space="PSUM") as psum_wg:
            wgps = [None] * n_geo
            gps = [None] * NGRP

            def emit_group(j):
                kind, ob = groups[j]
                ps = psum_p.tile([P, N], F32, tag="pps")
                if kind == "q":
                    for kb in range(NRB):
                        nc.tensor.matmul(ps[:],
                                         wsl(0, kb, ob * P, (ob + 1) * P),
                                         xqT(kb),
                                         start=(kb == 0),
                                         stop=(kb == NRB - 1))
                elif kind == "k":
                    for kb in range(NRB):
                        nc.tensor.matmul(ps[:, :K5pad],
                                         wsl(1, kb, ob * P, (ob + 1) * P),
                                         xkT(kb),
                                         start=(kb == 0),
                                         stop=(kb == NRB - 1))
                else:
                    for kb in range(NRB):
                        nc.tensor.matmul(ps[:], xvT(kb, ob * P, (ob + 1) * P),
                                         wsl(2, kb, 0, D),
                                         start=(kb == 0), stop=False)
                    nc.tensor.matmul(ps[:], ones33_bf[0:1, :], bvrow,
                                     start=False, stop=True)
                gps[j] = ps

            def evict_group(j):
                kind, ob = groups[j]
                ps = gps[j]
                if kind == "q":
                    nc.scalar.activation(qT[:, ob, :], ps[:], AF.Identity,
                                         bias=bq_t[:, ob:ob + 1])
                elif kind == "k":
                    nc.scalar.activation(kTt[:, ob, :], ps[:, :K5pad],
                                         AF.Identity, scale=0.125,
                                         bias=bk8x_t[:, ob:ob + 1])
                else:
                    nc.scalar.copy(v_sb[:, ob, :], ps[:])

            # distribute groups over geo iters (emit_group(j) at iter sched[j])
            sched = {}
            g0 = max(1, n_geo // 3)
            span = max(1, n_geo - g0)
            for j in range(NGRP):
                sched.setdefault(min(g0 + j * span // NGRP, n_geo - 1),
                                 []).append(j)

            def stage_b(g):
                wgdB = work4.tile([P, Q5pad], BF16, tag="wgdB")
                nc.vector.tensor_scalar(wgdB[:], wgps[g][:, :Q5pad],
                                        bgm1_t[:, g:g + 1],
                                        epsm1_t[:, g:g + 1],
                                        ALU.add, ALU.max)
                wgdm1 = work4.tile([P, Q5pad], BF16, tag="wgdm1")
                nc.gpsimd.tensor_tensor(wgdm1[:], wgdB[:], objqbc[:],
                                        ALU.mult)
                wgdM = work4.tile([P, Q5pad], BF16, tag="wgdM")
                nc.gpsimd.tensor_scalar(wgdM[:], wgdm1[:], 1.0, None, ALU.add)
                nc.scalar.dma_start(
                    wgd_dram[g].rearrange("t h q -> h t q"), wgdM[:])

            prev_groups = []
            for g in range(n_geo):
                blk = g // 8
                off = 64 * ((g % 8) // 4)
                q4 = g % 4
                mbase = g * GM
                lhs_wh = work4.tile([64, P], BF16, tag="lhs_wh")
                nc.vector.tensor_tensor(
                    lhs_wh[:].rearrange("k (h m) -> k h m", h=H),
                    w1e_f.rearrange("k (h m) -> k h m", h=H),
                    bankM[:, mbase:mbase + GM][:, None, :]
                        .to_broadcast((64, H, GM)),
                    ALU.mult)
                ups = psum_u.tile([P, 2, N], F32, tag="ups")
                for ci in range(2):
                    nc.tensor.matmul(ups[:, ci, :Q5pad],
                                     selap(off, off + 64, q4),
                                     dxy2[off:off + 64, blk, ci, :],
                                     start=True, stop=True)
                # projections fill the PE gap while DVE folds
                for j in sched.get(g, []):
                    emit_group(j)
                upsv = ups[:, :, :Q5pad]
                rrS = work4.tile([P, 2, Q5pad], F32, tag="rrS")
                nc.vector.tensor_scalar(rrS[:], upsv, MAGIC, -MAGIC,
                                        ALU.add, ALU.add)
                nfS = work4.tile([P, 2, Q5pad], F32, tag="nfS")
                nc.vector.tensor_tensor(nfS[:], rrS[:], upsv, ALU.subtract)
                # cos fold from nfS: nfC = nfS + (nfS <= -0.25)
                ind = work4.tile([P, 2, Q5pad], F32, tag="ind")
                nc.vector.tensor_scalar(ind[:], nfS[:], -0.25, None,
                                        ALU.is_le)
                nfC = work4.tile([P, 2, Q5pad], F32, tag="nfC")
                nc.vector.tensor_tensor(nfC[:], nfS[:], ind[:], ALU.add)
                sS = work4.tile([P, 2, Q5pad], BF16, tag="sS")
                nc.scalar.activation(sS[:], nfS[:], AF.Sin, scale=-TWO_PI)
                sC = work4.tile([P, 2, Q5pad], BF16, tag="sC")
                nc.scalar.activation(sC[:], nfC[:], AF.Sin, scale=-TWO_PI,
                                     bias=halfpi_t[:])
                wgp = psum_wg.tile([P, N], F32, tag="wgp")
                nc.tensor.matmul(wgp[:, :Q5pad], wblk(0), sS[:, 0, :],
                                 start=True, stop=False)
                nc.tensor.matmul(wgp[:, :Q5pad], wblk(1), sC[:, 0, :],
                                 start=False, stop=False)
                nc.tensor.matmul(wgp[:, :Q5pad], wblk(2), sS[:, 1, :],
                                 start=False, stop=False)
                nc.tensor.matmul(wgp[:, :Q5pad], wblk(3), sC[:, 1, :],
                                 start=False, stop=False)
                nc.tensor.matmul(wgp[:, :Q5pad], lhs_wh[:], bankN[:],
                                 start=False, stop=True)
                wgps[g] = wgp
                if g >= 1:
                    stage_b(g - 1)
                for j in prev_groups:
                    evict_group(j)
                prev_groups = sched.get(g, [])
            stage_b(n_geo - 1)
            for j in prev_groups:
                evict_group(j)
            # gather to attention layout: one DMA per block
            for blk in range(GBLK):
                gcnt = min(8, n_geo - blk * 8)
                nc.scalar.dma_start(
                    wgdT[0:gcnt * GM, blk, :, :],
                    wgd_dram[blk * 8:blk * 8 + gcnt]
                        .rearrange("g t h q -> (g t) h q"))

        # ---------------- phase 5: attention (2-stage pipeline) -------------
        ot = persist.tile([P, NRB, N], BF16, tag="ot")
        with tc.tile_pool(name="work5", bufs=4) as work5, \
             tc.tile_pool(name="psum5", bufs=2, space="PSUM") as psum5, \
             tc.tile_pool(name="psum_s", bufs=1, space="PSUM") as psum_s, \
             tc.tile_pool(name="psum_r", bufs=1, space="PSUM") as psum_r, \
             tc.tile_pool(name="psum_av", bufs=1, space="PSUM") as psum_av:
            flat = [(ob, rb) for ob in range(NRB) for rb in range(NRB5)]
            avs, sbanks, es = {}, {}, {}

            def accum(ob, rb):
                h0 = 2 * ob
                if rb == 0:
                    av_t = psum_av.tile([P, N], F32, tag="avps")
                    sb_t = psum_s.tile([2, N], F32, tag="sbank")
                    avs[ob] = av_t
                    sbanks[ob] = sb_t
                e_ = es[(ob, rb)]
                for hi in range(2):
                    po = hi * DK
                    nc.tensor.matmul(sbanks[ob][:], oh2[:, 2 * hi:2 * hi + 2],
                                     e_[:, hi, :],
                                     start=(rb == 0 and hi == 0),
                                     stop=(rb == NRB5 - 1 and hi == 1),
                                     skip_group_check=True)
                    nc.tensor.matmul(avs[ob][po:po + DK, :],
                                     v_sb[:, rb,
                                          (h0 + hi) * DK:(h0 + hi + 1) * DK],
                                     e_[:, hi, :], start=(rb == 0),
                                     stop=(rb == NRB5 - 1),
                                     skip_group_check=True)

            def finish(ob):
                rs = work5.tile([2, N], F32, tag="rs")
                nc.vector.reciprocal(rs[:], sbanks[ob][:])
                rrp = psum_r.tile([P, N], F32, tag="rrp")
                nc.tensor.matmul(rrp[:], bc8_t, rs[:], start=True, stop=True)
                rr_b = work5.tile([P, N], F32, tag="rr_b")
                nc.scalar.copy(rr_b[:], rrp[:])
                nc.vector.tensor_tensor(ot[:, ob, :], avs[ob][:], rr_b[:],
                                        ALU.mult)

            for i, (ob, rb) in enumerate(flat):
                h0 = 2 * ob
                st2 = psum5.tile([P, 2, N], F32, tag="stps")
                for hi in range(2):
                    po = hi * DK
                    nc.tensor.matmul(
                        st2[:, hi, :],
                        kTt[po:po + DK, ob, rb * P:(rb + 1) * P],
                        qT[po:po + DK, ob, :], start=True, stop=True)
                e_ = work5.tile([P, 2, N], BF16, tag="e_t")
                nc.scalar.activation(e_[:], st2[:], AF.Exp,
                                     bias=mcol_t[:, rb:rb + 1])
                if rb < GBLK:
                    rows = min(P, G5pad - rb * P)
                    nc.vector.tensor_tensor(
                        e_[0:rows, :, 0:Q5pad], e_[0:rows, :, 0:Q5pad],
                        wgdT[0:rows, rb, h0:h0 + 2, :], ALU.mult)
                es[(ob, rb)] = e_
                if i >= 2:
                    accum(*flat[i - 2])
                    if flat[i - 2][1] == NRB5 - 1:
                        finish(flat[i - 2][0])
            for i in (len(flat) - 2, len(flat) - 1):
                accum(*flat[i])
                if flat[i][1] == NRB5 - 1:
                    finish(flat[i][0])

        # ---------------- phase 6: output projection ----------------
        with tc.tile_pool(name="work6", bufs=2) as work6, \
             tc.tile_pool(name="psum6", bufs=2, space="PSUM") as psum6:
            fps = []

            def evict(rr):
                fo = work6.tile([P, D], F32, tag="fo")
                if rr % 2 == 0:
                    nc.scalar.copy(fo[:], fps[rr][:])
                else:
                    nc.vector.tensor_copy(fo[:], fps[rr][:])
                nc.sync.dma_start(out[rr * P:(rr + 1) * P, :], fo[:])

            for r in range(NRB):
                ps = psum6.tile([P, D], F32, tag="fps")
                for kt in range(NRB):
                    nc.tensor.matmul(ps[:], ot[:, kt, r * P:(r + 1) * P],
                                     wsl(3, kt, 0, D),
                                     start=(kt == 0), stop=False)
                nc.tensor.matmul(ps[:], ones33_bf[32:33, :], borow,
                                 start=False, stop=True)
                fps.append(ps)
                if r >= 1:
                    evict(r - 1)
            evict(NRB - 1)

    _split_multi_waits(nc)
    return nc


_NC_CACHE = {}


def kernel(**inputs):
    in_maps, sizes, inv_q = _host_prep(inputs)
    if _NC_CACHE.get("sizes") != sizes:
        _NC_CACHE["nc"] = build_nc(*sizes)
        _NC_CACHE["sizes"] = sizes
    nc = _NC_CACHE["nc"]
    res = run_bass_kernel_spmd(nc, in_maps, list(range(B)))
    out = np.stack([res.results[b]["out"][inv_q[b]] for b in range(B)], axis=0)
    return out.astype(np.float32)


if __name__ == "__main__":
    print("kernel module ok")


# revision 18
# speedup vs baseline: 1.0107x; 1.0071x over previous
"""Trainium2 Bass kernel for BoxMultiHeadedAttention (B=8, N=512, D=512, H=8).

Sharding: data-parallel over batch — each of the 8 NeuronCores computes one
batch element end-to-end; weights replicated; no collectives.

Sparsity compaction (host-side, per call; sizes padded to the max over the
8 batch elements so a single SPMD program serves all cores):
  * keys with mask==0 contribute exp(-1e9)=0 -> dropped entirely;
    kept keys ordered [mask&obj ("geo" keys) | mask&~obj], padded to
    NRB5*128 with -1e9 mask columns.
  * queries permuted obj-first: the geometry bias only applies to
    (obj_i & obj_j) pairs, so wg is computed for geo-keys x obj-queries
    only; per-core residual regions are neutralized with data
    ([P,1] bias/clip vectors and an obj-query column mask).
  * output rows are inverse-permuted on the host.

Per-core algorithm (layout [keys(part), queries(free)] throughout):
  * x shipped bf16 as one concatenated [xq;xk;xv] matrix -> 4 XBAR
    DMA-transposes; all weights in one packed DMA; all f32/bf16 consts in
    one packed DMA each (DMA issue is latency-chained, so count matters).
  * geometry: g = clip(ln((dx/w_i)^2), C2) on DVE+ACT; phases t = a/(4pi)*g
    via f32 selector matmul; sin/cos by exact magic-number folds
    (sin(2pi t) = Sin(-2pi*(round(t)-t)); cos via round(t+1/4) and
    bias pi/2); per-head contraction on PE (bf16); dw/dh separable
    rank-64 bank contraction.
  * wg multiplier M = 1 + max(wg+bG-1, 1e-6-1)*objq masked per-core via
    [P,1] vectors; routed to attention layout through a DRAM roundtrip
    (plain-SBUF DMAs; the (h,g) permutation lives in DRAM-side APs).
  * exp-domain softmax: T = E * M on the geo sub-tile only; row sums via
    ones-matmul; 1/rowsum broadcast across partitions by an exact f32
    selection matmul (no DRAM roundtrip); output projection bias folded
    in as a ones-row matmul.
All loops are software-pipelined (finalization of iter g emitted after the
start of iter g+1) so the in-order engine queues never head-of-line block.
"""
import math
import numpy as np
import ml_dtypes
from contextlib import ExitStack

import concourse.bass as bass
import concourse.mybir as mybir
import concourse.tile as tile
from concourse.bass_utils import run_bass_kernel_spmd

F32 = mybir.dt.float32
BF16 = mybir.dt.bfloat16
AF = mybir.ActivationFunctionType
ALU = mybir.AluOpType

B, N, D, H = 8, 512, 512, 8
DK = D // H
P = 128
NRB = N // P
GM = 16
WAVE_LEN = 1000.0
MAGIC = 12582912.0
C2 = float(2.0 * math.log(0.001))
ESHIFT = -6.0
TWO_PI = float(2.0 * math.pi)
HALF_PI = float(math.pi / 2.0)

_alphas = (100.0 / (WAVE_LEN ** (np.arange(8) / 8.0))).astype(np.float64)
BF = ml_dtypes.bfloat16


def _split_multi_waits(nc):
    """walrus accepts only ONE sync-wait per ISA instruction; hoist extras
    onto NoOps inserted before the offending instruction."""
    n_fix = 0
    for blk in nc.main_func.blocks:
        insts = list(blk.instructions)
        out, dirty = [], False
        for inst in insts:
            si = inst.sync_info
            waits = list(si.on_wait) if si is not None else []
            if len(waits) > 1:
                for kk, w in enumerate(waits[:-1]):
                    out.append(mybir.InstNoOp(
                        name=f"I-waitfix-{n_fix}-{kk}", engine=inst.engine,
                        sync_info=mybir.SyncInfo(on_wait=[w], on_update=[])))
                inst.sync_info = mybir.SyncInfo(
                    on_wait=[waits[-1]], on_update=list(si.on_update))
                n_fix += 1
                dirty = True
            out.append(inst)
        if dirty:
            blk.instructions = out
    return n_fix


def _selector_const():
    # SELAP[64*W + q*16 + m_loc, q*128 + m_loc*8 + j] = alpha_j/(4pi)
    selap = np.zeros((P, 4, P), dtype=np.float32)
    for W in range(2):
        for q in range(4):
            for m_loc in range(GM):
                for j in range(8):
                    selap[64 * W + q * 16 + m_loc, q, m_loc * 8 + j] = \
                        _alphas[j] / (4.0 * math.pi)
    return selap.reshape(P, 4 * P)


def _onehot8():
    oh = np.zeros((P, H, H), dtype=np.float32)
    for h in range(H):
        oh[:, h, h] = 1.0
    return oh.reshape(P, H * H)


def _wblk_direct(WG):
    # direct sin/cos weights: c in (sin-x, cos-x, sin-y, cos-y)
    gmap = [lambda j: j, lambda j: 32 + j, lambda j: 8 + j, lambda j: 40 + j]
    wblk = np.zeros((P, 4, P), dtype=np.float32)
    for c in range(4):
        for m_loc in range(GM):
            for j in range(8):
                for h in range(H):
                    wblk[m_loc * 8 + j, c, h * GM + m_loc] = WG[h, gmap[c](j)]
    return wblk.reshape(P, 4 * P)


def _bank_consts(WG):
    # dw/dh rank-64 decomposition (sin(A-B) via quarter-phase shifts)
    acol = np.zeros((64, 1), np.float32)
    pcol_m = np.zeros((64, 1), np.float32)
    pcol_n = np.zeros((64, 1), np.float32)
    w1 = np.zeros((64, H), np.float32)
    for f in range(2):
        for j in range(8):
            gs = 16 + 8 * f + j
            gc = 48 + 8 * f + j
            a = _alphas[j] / (4.0 * math.pi)
            for t in range(4):
                k = (f * 8 + j) * 4 + t
                acol[k, 0] = a
                pcol_m[k, 0] = 0.25 if t in (0, 2) else 0.0
                if t == 0:
                    pcol_n[k, 0] = 0.0; w1[k] = WG[:, gs]
                elif t == 1:
                    pcol_n[k, 0] = 0.75; w1[k] = WG[:, gs]   # -cos -> +pi
                elif t == 2:
                    pcol_n[k, 0] = 0.25; w1[k] = WG[:, gc]
                else:
                    pcol_n[k, 0] = 0.0; w1[k] = WG[:, gc]
    w1e = np.repeat(w1, GM, axis=1).astype(np.float32)
    return acol, pcol_m, pcol_n, w1e


def _bc8_const():
    # rr_b[p, n] = rs2[p//64, n]: bc8[k, p] = 1 iff k == p//64
    bc8 = np.zeros((2, P), np.float32)
    for p_ in range(P):
        bc8[p_ // 64, p_] = 1.0
    return bc8


def _host_prep(inputs):
    q = np.asarray(inputs["input_query"], np.float32)
    k = np.asarray(inputs["input_key"], np.float32)
    v = np.asarray(inputs["input_value"], np.float32)
    box = np.asarray(inputs["input_box"], np.float32)
    mask = np.asarray(inputs["mask"])
    nobj = np.asarray(inputs["not_objects"])
    WG = np.asarray(inputs["WG"], np.float32)
    bG = np.asarray(inputs["bG"], np.float32)

    x_min, y_min, x_max, y_max = [box[..., i] for i in range(4)]
    cx = (x_min + x_max) * 0.5
    cy = (y_min + y_max) * 0.5
    ww = x_max - x_min + 1.0
    hh = y_max - y_min + 1.0
    l2w = (2.0 * np.log(ww)).astype(np.float32)
    l2h = (2.0 * np.log(hh)).astype(np.float32)

    keyo, qo, G5s, K5s, Q5s = [], [], [], [], []
    for b in range(B):
        m_b = mask[b] != 0
        o_b = ~nobj[b]
        geo = np.where(m_b & o_b)[0]
        oth = np.where(m_b & ~o_b)[0]
        keyo.append(np.concatenate([geo, oth]))
        qobj = np.where(o_b)[0]
        qrest = np.where(~o_b)[0]
        qo.append(np.concatenate([qobj, qrest]))
        G5s.append(len(geo)); K5s.append(len(geo) + len(oth))
        Q5s.append(len(qobj))

    G5max = max(max(G5s), 1)
    n_geo = (G5max + GM - 1) // GM
    G5pad = n_geo * GM
    GBLK = (G5pad + P - 1) // P
    K5max = max(max(K5s), 1)
    NRB5 = (K5max + P - 1) // P
    K5pad = NRB5 * P
    Q5max = max(max(Q5s), 1)
    Q5pad = min(N, ((Q5max + 31) // 32) * 32)
    sizes = (n_geo, GBLK, NRB5, Q5pad)
    CW = 4 + 4 + NRB5 + GBLK + GBLK + n_geo + n_geo

    acol, pcol_m, pcol_n, w1e = _bank_consts(WG)
    # shared bf16 pack: oh8(64) | wblk(512) | bvbo(512)
    b16p = np.zeros((P, 64 + 512 + 512), np.float32)
    b16p[:, 0:64] = _onehot8()
    b16p[:, 64:576] = _wblk_direct(WG)
    b16p[0, 576:1088] = np.asarray(inputs["bv"], np.float32)
    b16p[32, 576:1088] = np.asarray(inputs["bo"], np.float32)
    wall = np.concatenate([
        np.asarray(inputs["Wq"], np.float32),
        np.asarray(inputs["Wk"], np.float32),
        np.asarray(inputs["Wv"], np.float32),
        np.asarray(inputs["Wo"], np.float32)], axis=1)  # [512, 2048]
    shared = {
        "wall": wall.astype(BF),
        "b16p": b16p.astype(BF),
    }
    bqc = np.asarray(inputs["bq"], np.float32).reshape(NRB, P).T
    bk8c = np.asarray(inputs["bk"], np.float32).reshape(NRB, P).T

    # f32 pack layout: selap(512) | cpack(CW) | w1e(128) | cp64(3) | bc8(512)
    F32W = 512 + CW + 128 + 3 + 128
    f32_base = np.zeros((P, F32W), np.float32)
    f32_base[:, 0:512] = _selector_const()
    f32_base[0:64, 512 + CW:512 + CW + 128] = w1e
    f32_base[0:64, 512 + CW + 128:512 + CW + 131] = \
        np.concatenate([acol, pcol_m, pcol_n], axis=1)
    f32_base[0:2, 512 + CW + 131:512 + CW + 259] = _bc8_const()

    in_maps = []
    for b in range(B):
        ko, qp = keyo[b], qo[b]
        G5, K5, Q5 = G5s[b], K5s[b], Q5s[b]

        xall = np.zeros((N + 2 * K5pad, D), BF)
        xall[0:N] = q[b][qp].astype(BF)
        xall[N:N + K5] = k[b][ko].astype(BF)
        xall[N + K5pad:N + K5pad + K5] = v[b][ko].astype(BF)

        cxk = np.zeros(GBLK * P, np.float32); cxk[:G5] = cx[b][ko[:G5]]
        cyk = np.zeros(GBLK * P, np.float32); cyk[:G5] = cy[b][ko[:G5]]
        l2wk = np.zeros(G5pad, np.float32); l2wk[:G5] = l2w[b][ko[:G5]]
        l2hk = np.zeros(G5pad, np.float32); l2hk[:G5] = l2h[b][ko[:G5]]
        bq4 = np.zeros((4, Q5pad), np.float32)
        bq4[2:] = 1.0
        l2q2 = np.zeros((2, Q5pad), np.float32)
        nq = min(Q5pad, N)
        bq4[0, :nq] = cx[b][qp[:nq]]; bq4[1, :nq] = cy[b][qp[:nq]]
        bq4[2, :nq] = 1.0 / ww[b][qp[:nq]]; bq4[3, :nq] = 1.0 / hh[b][qp[:nq]]
        l2q2[0, :nq] = l2w[b][qp[:nq]]; l2q2[1, :nq] = l2h[b][qp[:nq]]
        objq = np.zeros(Q5pad, np.float32)
        objq[:min(Q5, Q5pad)] = 1.0

        maskcol = np.full(NRB5 * P, -1e9 + ESHIFT, np.float32)
        maskcol[:K5] = ESHIFT
        maskcol = maskcol.reshape(NRB5, P).T

        bgm1 = np.zeros((P, n_geo), np.float32)
        epsm1 = np.zeros((P, n_geo), np.float32)
        for g in range(n_geo):
            for m in range(GM):
                key = g * GM + m
                for h in range(H):
                    if key < G5:
                        bgm1[h * GM + m, g] = bG[h] - 1.0
                        epsm1[h * GM + m, g] = 1e-6 - 1.0
                    else:
                        bgm1[h * GM + m, g] = -1e9
                        epsm1[h * GM + m, g] = 0.0

        f32p = f32_base.copy()
        f32p[:, 512:512 + CW] = np.concatenate([
            bqc, bk8c, maskcol,
            cxk.reshape(GBLK, P).T, cyk.reshape(GBLK, P).T,
            bgm1, epsm1], axis=1)

        l2kM = np.concatenate([np.broadcast_to(l2wk, (32, G5pad)),
                               np.broadcast_to(l2hk, (32, G5pad))], axis=0)
        l2qNh = np.concatenate([np.broadcast_to(l2q2[0], (32, Q5pad)),
                                np.broadcast_to(l2q2[1], (32, Q5pad))], axis=0)

        mm = dict(shared)
        mm.update({
            "xall": xall,
            "f32p": np.ascontiguousarray(f32p),
            "bq4": bq4,
            "l2kM": np.ascontiguousarray(l2kM),
            "l2qN": np.ascontiguousarray(l2qNh),
            "objq": objq.astype(BF),
        })
        in_maps.append(mm)

    inv_q = [np.argsort(qp) for qp in qo]
    return in_maps, sizes, inv_q


def build_nc(n_geo, GBLK, NRB5, Q5pad):
    K5pad = NRB5 * P
    G5pad = n_geo * GM
    XR = N + 2 * K5pad
    CW = 4 + 4 + NRB5 + GBLK + GBLK + n_geo + n_geo
    F32W = 512 + CW + 128 + 3 + 128
    nc = bass.Bass()

    def dp(name, shape, dt=F32):
        return nc.declare_dram_parameter(name, list(shape), dt, isOutput=False)

    XALL = dp("xall", (XR, D), BF16)
    WALL = dp("wall", (D, 4 * D), BF16)
    F32P = dp("f32p", (P, F32W))
    B16P = dp("b16p", (P, 1088), BF16)
    BQ4 = dp("bq4", (4, Q5pad))
    L2KM = dp("l2kM", (64, G5pad))
    L2QN = dp("l2qN", (64, Q5pad))
    OBJQ = dp("objq", (Q5pad,), BF16)
    out = nc.declare_dram_parameter("out", [N, D], F32, isOutput=True)
    wgd_dram = nc.dram_tensor("wgd_scratch", [n_geo, GM, H, Q5pad], BF16)

    with ExitStack() as ctx:
        tc = ctx.enter_context(tile.TileContext(nc))
        const = ctx.enter_context(tc.tile_pool(name="const", bufs=1))
        persist = ctx.enter_context(tc.tile_pool(name="persist", bufs=1))

        # ------------- constants (ACT queue) --------------------------------
        f32p = const.tile([P, F32W], F32, tag="f32p")
        nc.scalar.dma_start(f32p[:], F32P[:])
        CPo = 512
        W1o = 512 + CW
        C64o = W1o + 128
        BC8o = C64o + 3
        bq_t = f32p[:, CPo:CPo + 4]
        bk8x_t = f32p[:, CPo + 4:CPo + 8]
        mcol_t = f32p[:, CPo + 8:CPo + 8 + NRB5]
        cxk_t = f32p[:, CPo + 8 + NRB5:CPo + 8 + NRB5 + GBLK]
        cyk_t = f32p[:, CPo + 8 + NRB5 + GBLK:CPo + 8 + NRB5 + 2 * GBLK]
        bg_o = CPo + 8 + NRB5 + 2 * GBLK
        bgm1_t = f32p[:, bg_o:bg_o + n_geo]
        epsm1_t = f32p[:, bg_o + n_geo:bg_o + 2 * n_geo]
        w1e_f = f32p[0:64, W1o:W1o + 128]
        acol_t = f32p[0:64, C64o:C64o + 1]
        pcolm_t = f32p[0:64, C64o + 1:C64o + 2]
        pcoln_t = f32p[0:64, C64o + 2:C64o + 3]

        def selap(r0, r1, q4):
            return f32p[r0:r1, q4 * P:(q4 + 1) * P]

        bc8_t = f32p[0:2, BC8o:BC8o + P]

        b16p = const.tile([P, 1088], BF16, tag="b16p")
        nc.scalar.dma_start(b16p[:], B16P[:])

        def oh8(h):
            return b16p[:, h * 8:(h + 1) * 8]

        def wblk(c):
            return b16p[:, 64 + c * P:64 + (c + 1) * P]
        bvrow = b16p[0:1, 576:1088]
        borow = b16p[32:33, 576:1088]

        bq4bc = const.tile([P, 4, Q5pad], F32, tag="bq4bc")
        nc.scalar.dma_start(bq4bc[:],
                            BQ4[None, :, :].to_broadcast((P, 4, Q5pad)))
        cxqbc = bq4bc[:, 0, :]; cyqbc = bq4bc[:, 1, :]
        iwqbc = bq4bc[:, 2, :]; ihqbc = bq4bc[:, 3, :]
        objqbc = const.tile([P, Q5pad], BF16, tag="objqbc")
        nc.scalar.dma_start(objqbc[:], OBJQ[None, :].to_broadcast((P, Q5pad)))
        l2kM = const.tile([64, G5pad], F32, tag="l2kM")
        nc.scalar.dma_start(l2kM[:], L2KM[:])
        l2qN = const.tile([64, Q5pad], F32, tag="l2qN")
        nc.scalar.dma_start(l2qN[:], L2QN[:])
        halfpi_t = const.tile([P, 1], F32, tag="halfpi")
        nc.vector.memset(halfpi_t[:], HALF_PI)
        oh2 = const.tile([P, 4], BF16, tag="oh2")
        nc.vector.memset(oh2[:], 0.0)
        nc.vector.memset(oh2[:, 0:1], 1.0)
        nc.vector.memset(oh2[:, 3:4], 1.0)
        ones33_bf = const.tile([33, P], BF16, tag="ones33")
        nc.vector.memset(ones33_bf[:], 1.0)

        # ------------- input loads first (PE critical path, SP queue) -------
        xallT = persist.tile([P, NRB, XR], BF16, tag="xallT")
        wall = persist.tile([P, NRB, 4 * D], BF16, tag="wall")
        for cb in range(NRB):
            nc.sync.dma_start_transpose(xallT[:, cb, :],
                                        XALL[:, cb * P:(cb + 1) * P])
        nc.sync.dma_start(wall[:], WALL.rearrange("(kb p) d -> p kb d", p=P))

        def xqT(kb):
            return xallT[:, kb, 0:N]

        def xkT(kb):
            return xallT[:, kb, N:N + K5pad]

        def xvT(kb, c0, c1):
            return xallT[:, kb, N + K5pad + c0:N + K5pad + c1]

        def wsl(wi, kb, c0, c1):
            return wall[:, kb, wi * D + c0:wi * D + c1]

        # ---------------- phase 2: ln fields (pipelined) --------------------
        qT = persist.tile([P, NRB, N], BF16, tag="qT")
        kTt = persist.tile([P, NRB, K5pad], BF16, tag="kT")
        v_sb = persist.tile([P, NRB5, D], BF16, tag="v_sb")
        dxy2 = persist.tile([P, GBLK, 2, Q5pad], F32, tag="dxy2")
        with tc.tile_pool(name="work2", bufs=5) as work2:
            items = [(blk, ci) for blk in range(GBLK) for ci in range(2)]
            dws = {}
            l2s = {}
            for (blk, ci) in items:
                cbc = cxqbc if ci == 0 else cyqbc
                ccol = cxk_t if ci == 0 else cyk_t
                ibc = iwqbc if ci == 0 else ihqbc
                d_ = work2.tile([P, Q5pad], F32, tag="geo_d")
                nc.vector.tensor_scalar(d_[:], cbc, ccol[:, blk:blk + 1],
                                        None, ALU.subtract)
                dw_ = work2.tile([P, Q5pad], F32, tag="geo_dw")
                nc.vector.tensor_tensor(dw_[:], d_[:], ibc, ALU.mult)
                dws[(blk, ci)] = dw_
            for (blk, ci) in items:
                d2 = work2.tile([P, Q5pad], F32, tag="geo_d2")
                nc.scalar.activation(d2[:], dws[(blk, ci)][:], AF.Square)
                l2t = work2.tile([P, Q5pad], F32, tag="geo_l2")
                nc.scalar.activation(l2t[:], d2[:], AF.Ln)
                l2s[(blk, ci)] = l2t
            for (blk, ci) in items:
                nc.vector.tensor_scalar_max(dxy2[:, blk, ci, :],
                                            l2s[(blk, ci)][:], C2)

        # ---------------- phase 3: dw/dh banks ----------------
        bankM = persist.tile([64, G5pad], BF16, tag="bankM")
        bankN = persist.tile([64, Q5pad], BF16, tag="bankN")
        with tc.tile_pool(name="work3", bufs=2) as work3:
            bk_items = ((pcolm_t, l2kM, G5pad, bankM),
                        (pcoln_t, l2qN, Q5pad, bankN))
            fs = []
            for (pcol, l2bc, width, bank) in bk_items:
                t_ = work3.tile([64, width], F32, tag="bk_t")
                nc.vector.tensor_scalar(t_[:], l2bc[:], acol_t, pcol,
                                        ALU.mult, ALU.add)
                r_ = work3.tile([64, width], F32, tag="bk_r")
                nc.vector.tensor_scalar(r_[:], t_[:], MAGIC, -MAGIC,
                                        ALU.add, ALU.add)
                f_ = work3.tile([64, width], F32, tag="bk_f")
                nc.vector.tensor_tensor(f_[:], t_[:], r_[:], ALU.subtract)
                fs.append(f_)
            for (f_, (pcol, l2bc, width, bank)) in zip(fs, bk_items):
                nc.scalar.activation(bank[:], f_[:], AF.Sin, scale=TWO_PI)

        # ------- phase 4: geometry weights + interleaved projections --------
        # proj groups: (kind, idx); evictions on ACT so DVE stays on folds
        groups = ([("q", ob) for ob in range(NRB)]
                  + [("k", ob) for ob in range(NRB)]
                  + [("v", mb) for mb in range(NRB5)])
        NGRP = len(groups)

        wgdT = persist.tile([P, GBLK, H, Q5pad], BF16, tag="wgdT")
        with tc.tile_pool(name="work4", bufs=3) as work4, \
             tc.tile_pool(name="psum_u", bufs=2, space="PSUM") as psum_u, \
             tc.tile_pool(name="psum_p", bufs=2, space="PSUM") as psum_p, \
             tc.tile_pool(name="psum_wg", bufs=2, space="PSUM") as psum_wg:
            wgps = [None] * n_geo
            gps = [None] * NGRP

            def emit_group(j):
                kind, ob = groups[j]
                ps = psum_p.tile([P, N], F32, tag="pps")
                if kind == "q":
                    for kb in range(NRB):
                        nc.tensor.matmul(ps[:],
                                         wsl(0, kb, ob * P, (ob + 1) * P),
                                         xqT(kb),
                                         start=(kb == 0),
                                         stop=(kb == NRB - 1))
                elif kind == "k":
                    for kb in range(NRB):
                        nc.tensor.matmul(ps[:, :K5pad],
                                         wsl(1, kb, ob * P, (ob + 1) * P),
                                         xkT(kb),
                                         start=(kb == 0),
                                         stop=(kb == NRB - 1))
                else:
                    for kb in range(NRB):
                        nc.tensor.matmul(ps[:], xvT(kb, ob * P, (ob + 1) * P),
                                         wsl(2, kb, 0, D),
                                         start=(kb == 0), stop=False)
                    nc.tensor.matmul(ps[:], ones33_bf[0:1, :], bvrow,
                                     start=False, stop=True)
                gps[j] = ps

            def evict_group(j):
                kind, ob = groups[j]
                ps = gps[j]
                if kind == "q":
                    nc.scalar.activation(qT[:, ob, :], ps[:], AF.Identity,
                                         bias=bq_t[:, ob:ob + 1])
                elif kind == "k":
                    nc.scalar.activation(kTt[:, ob, :], ps[:, :K5pad],
                                         AF.Identity, scale=0.125,
                                         bias=bk8x_t[:, ob:ob + 1])
                else:
                    nc.scalar.copy(v_sb[:, ob, :], ps[:])

            # distribute groups over geo iters (emit_group(j) at iter sched[j])
            sched = {}
            for j in range(NGRP):
                sched.setdefault(min(j * n_geo // NGRP, n_geo - 1), []).append(j)

            def stage_b(g):
                wgdB = work4.tile([P, Q5pad], BF16, tag="wgdB")
                nc.vector.tensor_scalar(wgdB[:], wgps[g][:, :Q5pad],
                                        bgm1_t[:, g:g + 1],
                                        epsm1_t[:, g:g + 1],
                                        ALU.add, ALU.max)
                wgdm1 = work4.tile([P, Q5pad], BF16, tag="wgdm1")
                nc.gpsimd.tensor_tensor(wgdm1[:], wgdB[:], objqbc[:],
                                        ALU.mult)
                wgdM = work4.tile([P, Q5pad], BF16, tag="wgdM")
                nc.gpsimd.tensor_scalar(wgdM[:], wgdm1[:], 1.0, None, ALU.add)
                nc.scalar.dma_start(
                    wgd_dram[g].rearrange("t h q -> h t q"), wgdM[:])

            prev_groups = []
            for g in range(n_geo):
                blk = g // 8
                off = 64 * ((g % 8) // 4)
                q4 = g % 4
                mbase = g * GM
                lhs_wh = work4.tile([64, P], BF16, tag="lhs_wh")
                nc.vector.tensor_tensor(
                    lhs_wh[:].rearrange("k (h m) -> k h m", h=H),
                    w1e_f.rearrange("k (h m) -> k h m", h=H),
                    bankM[:, mbase:mbase + GM][:, None, :]
                        .to_broadcast((64, H, GM)),
                    ALU.mult)
                ups = psum_u.tile([P, 2, N], F32, tag="ups")
                for ci in range(2):
                    nc.tensor.matmul(ups[:, ci, :Q5pad],
                                     selap(off, off + 64, q4),
                                     dxy2[off:off + 64, blk, ci, :],
                                     start=True, stop=True)
                # projections fill the PE gap while DVE folds
                for j in sched.get(g, []):
                    emit_group(j)
                upsv = ups[:, :, :Q5pad]
                rrS = work4.tile([P, 2, Q5pad], F32, tag="rrS")
                nc.vector.tensor_scalar(rrS[:], upsv, MAGIC, -MAGIC,
                                        ALU.add, ALU.add)
                nfS = work4.tile([P, 2, Q5pad], F32, tag="nfS")
                nc.vector.tensor_tensor(nfS[:], rrS[:], upsv, ALU.subtract)
                # cos fold from nfS: nfC = nfS + (nfS <= -0.25)
                ind = work4.tile([P, 2, Q5pad], F32, tag="ind")
                nc.vector.tensor_scalar(ind[:], nfS[:], -0.25, None,
                                        ALU.is_le)
                nfC = work4.tile([P, 2, Q5pad], F32, tag="nfC")
                nc.vector.tensor_tensor(nfC[:], nfS[:], ind[:], ALU.add)
                sS = work4.tile([P, 2, Q5pad], BF16, tag="sS")
                nc.scalar.activation(sS[:], nfS[:], AF.Sin, scale=-TWO_PI)
                sC = work4.tile([P, 2, Q5pad], BF16, tag="sC")
                nc.scalar.activation(sC[:], nfC[:], AF.Sin, scale=-TWO_PI,
                                     bias=halfpi_t[:])
                wgp = psum_wg.tile([P, N], F32, tag="wgp")
                nc.tensor.matmul(wgp[:, :Q5pad], wblk(0), sS[:, 0, :],
                                 start=True, stop=False)
                nc.tensor.matmul(wgp[:, :Q5pad], wblk(1), sC[:, 0, :],
                                 start=False, stop=False)
                nc.tensor.matmul(wgp[:, :Q5pad], wblk(2), sS[:, 1, :],
                                 start=False, stop=False)
                nc.tensor.matmul(wgp[:, :Q5pad], wblk(3), sC[:, 1, :],
                                 start=False, stop=False)
                nc.tensor.matmul(wgp[:, :Q5pad], lhs_wh[:], bankN[:],
                                 start=False, stop=True)
                wgps[g] = wgp
                if g >= 1:
                    stage_b(g - 1)
                for j in prev_groups:
                    evict_group(j)
                prev_groups = sched.get(g, [])
            stage_b(n_geo - 1)
            for j in prev_groups:
                evict_group(j)
            # gather to attention layout: one DMA per block
            for blk in range(GBLK):
                gcnt = min(8, n_geo - blk * 8)
                nc.scalar.dma_start(
                    wgdT[0:gcnt * GM, blk, :, :],
                    wgd_dram[blk * 8:blk * 8 + gcnt]
                        .rearrange("g t h q -> (g t) h q"))

        # ---------------- phase 5: attention (2-stage pipeline) -------------
        ot = persist.tile([P, NRB, N], BF16, tag="ot")
        with tc.tile_pool(name="work5", bufs=4) as work5, \
             tc.tile_pool(name="psum5", bufs=2, space="PSUM") as psum5, \
             tc.tile_pool(name="psum_s", bufs=1, space="PSUM") as psum_s, \
             tc.tile_pool(name="psum_r", bufs=1, space="PSUM") as psum_r, \
             tc.tile_pool(name="psum_av", bufs=1, space="PSUM") as psum_av:
            flat = [(ob, rb) for ob in range(NRB) for rb in range(NRB5)]
            avs, sbanks, es = {}, {}, {}

            def accum(ob, rb):
                h0 = 2 * ob
                if rb == 0:
                    av_t = psum_av.tile([P, N], F32, tag="avps")
                    sb_t = psum_s.tile([2, N], F32, tag="sbank")
                    avs[ob] = av_t
                    sbanks[ob] = sb_t
                e_ = es[(ob, rb)]
                for hi in range(2):
                    po = hi * DK
                    nc.tensor.matmul(sbanks[ob][:], oh2[:, 2 * hi:2 * hi + 2],
                                     e_[:, hi, :],
                                     start=(rb == 0 and hi == 0),
                                     stop=(rb == NRB5 - 1 and hi == 1),
                                     skip_group_check=True)
                    nc.tensor.matmul(avs[ob][po:po + DK, :],
                                     v_sb[:, rb,
                                          (h0 + hi) * DK:(h0 + hi + 1) * DK],
                                     e_[:, hi, :], start=(rb == 0),
                                     stop=(rb == NRB5 - 1),
                                     skip_group_check=True)

            def finish(ob):
                rs = work5.tile([2, N], F32, tag="rs")
                nc.vector.reciprocal(rs[:], sbanks[ob][:])
                rrp = psum_r.tile([P, N], F32, tag="rrp")
                nc.tensor.matmul(rrp[:], bc8_t, rs[:], start=True, stop=True)
                rr_b = work5.tile([P, N], F32, tag="rr_b")
                nc.scalar.copy(rr_b[:], rrp[:])
                nc.vector.tensor_tensor(ot[:, ob, :], avs[ob][:], rr_b[:],
                                        ALU.mult)

            for i, (ob, rb) in enumerate(flat):
                h0 = 2 * ob
                st2 = psum5.tile([P, 2, N], F32, tag="stps")
                for hi in range(2):
                    po = hi * DK
                    nc.tensor.matmul(
                        st2[:, hi, :],
                        kTt[po:po + DK, ob, rb * P:(rb + 1) * P],
                        qT[po:po + DK, ob, :], start=True, stop=True)
                e_ = work5.tile([P, 2, N], BF16, tag="e_t")
                nc.scalar.activation(e_[:], st2[:], AF.Exp,
                                     bias=mcol_t[:, rb:rb + 1])
                if rb < GBLK:
                    rows = min(P, G5pad - rb * P)
                    nc.vector.tensor_tensor(
                        e_[0:rows, :, 0:Q5pad], e_[0:rows, :, 0:Q5pad],
                        wgdT[0:rows, rb, h0:h0 + 2, :], ALU.mult)
                es[(ob, rb)] = e_
                if i >= 2:
                    accum(*flat[i - 2])
                    if flat[i - 2][1] == NRB5 - 1:
                        finish(flat[i - 2][0])
            for i in (len(flat) - 2, len(flat) - 1):
                accum(*flat[i])
                if flat[i][1] == NRB5 - 1:
                    finish(flat[i][0])

        # ---------------- phase 6: output projection ----------------
        with tc.tile_pool(name="work6", bufs=2) as work6, \
             tc.tile_pool(name="psum6", bufs=2, space="PSUM") as psum6:
            fps = []

            def evict(rr):
                fo = work6.tile([P, D], F32, tag="fo")
                if rr % 2 == 0:
                    nc.scalar.copy(fo[:], fps[rr][:])
                else:
                    nc.vector.tensor_copy(fo[:], fps[rr][:])
                nc.sync.dma_start(out[rr * P:(rr + 1) * P, :], fo[:])

            for r in range(NRB):
                ps = psum6.tile([P, D], F32, tag="fps")
                for kt in range(NRB):
                    nc.tensor.matmul(ps[:], ot[:, kt, r * P:(r + 1) * P],
                                     wsl(3, kt, 0, D),
                                     start=(kt == 0), stop=False)
                nc.tensor.matmul(ps[:], ones33_bf[32:33, :], borow,
                                 start=False, stop=True)
                fps.append(ps)
                if r >= 1:
                    evict(r - 1)
            evict(NRB - 1)

    _split_multi_waits(nc)
    return nc


_NC_CACHE = {}


def kernel(**inputs):
    in_maps, sizes, inv_q = _host_prep(inputs)
    if _NC_CACHE.get("sizes") != sizes:
        _NC_CACHE["nc"] = build_nc(*sizes)
        _NC_CACHE["sizes"] = sizes
    nc = _NC_CACHE["nc"]
    res = run_bass_kernel_spmd(nc, in_maps, list(range(B)))
    out = np.stack([res.results[b]["out"][inv_q[b]] for b in range(B)], axis=0)
    return out.astype(np.float32)


if __name__ == "__main__":
    print("kernel module ok")


# revision 19
# speedup vs baseline: 1.0229x; 1.0120x over previous
"""Trainium2 Bass kernel for BoxMultiHeadedAttention (B=8, N=512, D=512, H=8).

Sharding: data-parallel over batch — each of the 8 NeuronCores computes one
batch element end-to-end; weights replicated; no collectives.

Sparsity compaction (host-side, per call; sizes padded to the max over the
8 batch elements so a single SPMD program serves all cores):
  * keys with mask==0 contribute exp(-1e9)=0 -> dropped entirely;
    kept keys ordered [mask&obj ("geo" keys) | mask&~obj], padded to
    NRB5*128 with -1e9 mask columns.
  * queries permuted obj-first: the geometry bias only applies to
    (obj_i & obj_j) pairs, so wg is computed for geo-keys x obj-queries
    only; per-core residual regions are neutralized with data
    ([P,1] bias/clip vectors and an obj-query column mask).
  * output rows are inverse-permuted on the host.

Per-core algorithm (layout [keys(part), queries(free)] throughout):
  * x shipped bf16 as one concatenated [xq;xk;xv] matrix -> 4 XBAR
    DMA-transposes; all weights in one packed DMA; all f32/bf16 consts in
    one packed DMA each (DMA issue is latency-chained, so count matters).
  * geometry: g = clip(ln((dx/w_i)^2), C2) on DVE+ACT; phases t = a/(4pi)*g
    via f32 selector matmul; sin/cos by exact magic-number folds
    (sin(2pi t) = Sin(-2pi*(round(t)-t)); cos via round(t+1/4) and
    bias pi/2); per-head contraction on PE (bf16); dw/dh separable
    rank-64 bank contraction.
  * wg multiplier M = 1 + max(wg+bG-1, 1e-6-1)*objq masked per-core via
    [P,1] vectors; routed to attention layout through a DRAM roundtrip
    (plain-SBUF DMAs; the (h,g) permutation lives in DRAM-side APs).
  * exp-domain softmax: T = E * M on the geo sub-tile only; row sums via
    ones-matmul; 1/rowsum broadcast across partitions by an exact f32
    selection matmul (no DRAM roundtrip); output projection bias folded
    in as a ones-row matmul.
All loops are software-pipelined (finalization of iter g emitted after the
start of iter g+1) so the in-order engine queues never head-of-line block.
"""
import math
import numpy as np
import ml_dtypes
from contextlib import ExitStack

import concourse.bass as bass
import concourse.mybir as mybir
import concourse.tile as tile
from concourse.bass_utils import run_bass_kernel_spmd

F32 = mybir.dt.float32
BF16 = mybir.dt.bfloat16
AF = mybir.ActivationFunctionType
ALU = mybir.AluOpType

B, N, D, H = 8, 512, 512, 8
DK = D // H
P = 128
NRB = N // P
GM = 16
WAVE_LEN = 1000.0
MAGIC = 12582912.0
C2 = float(2.0 * math.log(0.001))
ESHIFT = -6.0
TWO_PI = float(2.0 * math.pi)
HALF_PI = float(math.pi / 2.0)

_alphas = (100.0 / (WAVE_LEN ** (np.arange(8) / 8.0))).astype(np.float64)
BF = ml_dtypes.bfloat16


def _split_multi_waits(nc):
    """walrus accepts only ONE sync-wait per ISA instruction; hoist extras
    onto NoOps inserted before the offending instruction."""
    n_fix = 0
    for blk in nc.main_func.blocks:
        insts = list(blk.instructions)
        out, dirty = [], False
        for inst in insts:
            si = inst.sync_info
            waits = list(si.on_wait) if si is not None else []
            if len(waits) > 1:
                for kk, w in enumerate(waits[:-1]):
                    out.append(mybir.InstNoOp(
                        name=f"I-waitfix-{n_fix}-{kk}", engine=inst.engine,
                        sync_info=mybir.SyncInfo(on_wait=[w], on_update=[])))
                inst.sync_info = mybir.SyncInfo(
                    on_wait=[waits[-1]], on_update=list(si.on_update))
                n_fix += 1
                dirty = True
            out.append(inst)
        if dirty:
            blk.instructions = out
    return n_fix


def _selector_const():
    # SELAP[64*W + q*16 + m_loc, q*128 + m_loc*8 + j] = alpha_j/(4pi)
    selap = np.zeros((P, 4, P), dtype=np.float32)
    for W in range(2):
        for q in range(4):
            for m_loc in range(GM):
                for j in range(8):
                    selap[64 * W + q * 16 + m_loc, q, m_loc * 8 + j] = \
                        _alphas[j] / (4.0 * math.pi)
    return selap.reshape(P, 4 * P)


def _onehot8():
    oh = np.zeros((P, H, H), dtype=np.float32)
    for h in range(H):
        oh[:, h, h] = 1.0
    return oh.reshape(P, H * H)


def _wblk_direct(WG):
    # direct sin/cos weights: c in (sin-x, cos-x, sin-y, cos-y)
    gmap = [lambda j: j, lambda j: 32 + j, lambda j: 8 + j, lambda j: 40 + j]
    wblk = np.zeros((P, 4, P), dtype=np.float32)
    for c in range(4):
        for m_loc in range(GM):
            for j in range(8):
                for h in range(H):
                    wblk[m_loc * 8 + j, c, h * GM + m_loc] = WG[h, gmap[c](j)]
    return wblk.reshape(P, 4 * P)


def _bank_consts(WG):
    # dw/dh rank-64 decomposition (sin(A-B) via quarter-phase shifts)
    acol = np.zeros((64, 1), np.float32)
    pcol_m = np.zeros((64, 1), np.float32)
    pcol_n = np.zeros((64, 1), np.float32)
    w1 = np.zeros((64, H), np.float32)
    for f in range(2):
        for j in range(8):
            gs = 16 + 8 * f + j
            gc = 48 + 8 * f + j
            a = _alphas[j] / (4.0 * math.pi)
            for t in range(4):
                k = (f * 8 + j) * 4 + t
                acol[k, 0] = a
                pcol_m[k, 0] = 0.25 if t in (0, 2) else 0.0
                if t == 0:
                    pcol_n[k, 0] = 0.0; w1[k] = WG[:, gs]
                elif t == 1:
                    pcol_n[k, 0] = 0.75; w1[k] = WG[:, gs]   # -cos -> +pi
                elif t == 2:
                    pcol_n[k, 0] = 0.25; w1[k] = WG[:, gc]
                else:
                    pcol_n[k, 0] = 0.0; w1[k] = WG[:, gc]
    w1e = np.repeat(w1, GM, axis=1).astype(np.float32)
    return acol, pcol_m, pcol_n, w1e


def _bc8_const():
    # rr_b[p, n] = rs2[p//64, n]: bc8[k, p] = 1 iff k == p//64
    bc8 = np.zeros((2, P), np.float32)
    for p_ in range(P):
        bc8[p_ // 64, p_] = 1.0
    return bc8


def _host_prep(inputs):
    q = np.asarray(inputs["input_query"], np.float32)
    k = np.asarray(inputs["input_key"], np.float32)
    v = np.asarray(inputs["input_value"], np.float32)
    box = np.asarray(inputs["input_box"], np.float32)
    mask = np.asarray(inputs["mask"])
    nobj = np.asarray(inputs["not_objects"])
    WG = np.asarray(inputs["WG"], np.float32)
    bG = np.asarray(inputs["bG"], np.float32)

    x_min, y_min, x_max, y_max = [box[..., i] for i in range(4)]
    cx = (x_min + x_max) * 0.5
    cy = (y_min + y_max) * 0.5
    ww = x_max - x_min + 1.0
    hh = y_max - y_min + 1.0
    l2w = (2.0 * np.log(ww)).astype(np.float32)
    l2h = (2.0 * np.log(hh)).astype(np.float32)

    keyo, qo, G5s, K5s, Q5s = [], [], [], [], []
    for b in range(B):
        m_b = mask[b] != 0
        o_b = ~nobj[b]
        geo = np.where(m_b & o_b)[0]
        oth = np.where(m_b & ~o_b)[0]
        keyo.append(np.concatenate([geo, oth]))
        qobj = np.where(o_b)[0]
        qrest = np.where(~o_b)[0]
        qo.append(np.concatenate([qobj, qrest]))
        G5s.append(len(geo)); K5s.append(len(geo) + len(oth))
        Q5s.append(len(qobj))

    G5max = max(max(G5s), 1)
    n_geo = (G5max + GM - 1) // GM
    G5pad = n_geo * GM
    GBLK = (G5pad + P - 1) // P
    K5max = max(max(K5s), 1)
    NRB5 = (K5max + P - 1) // P
    K5pad = NRB5 * P
    Q5max = max(max(Q5s), 1)
    Q5pad = min(N, ((Q5max + 31) // 32) * 32)
    sizes = (n_geo, GBLK, NRB5, Q5pad)
    CW = 4 + 4 + NRB5 + GBLK + GBLK + n_geo + n_geo

    acol, pcol_m, pcol_n, w1e = _bank_consts(WG)
    # shared bf16 pack: oh8(64) | wblk(512) | bvbo(512)
    b16p = np.zeros((P, 64 + 512 + 512), np.float32)
    b16p[:, 0:64] = _onehot8()
    b16p[:, 64:576] = _wblk_direct(WG)
    b16p[0, 576:1088] = np.asarray(inputs["bv"], np.float32)
    b16p[32, 576:1088] = np.asarray(inputs["bo"], np.float32)
    wall = np.concatenate([
        np.asarray(inputs["Wq"], np.float32),
        np.asarray(inputs["Wk"], np.float32),
        np.asarray(inputs["Wv"], np.float32),
        np.asarray(inputs["Wo"], np.float32)], axis=1)  # [512, 2048]
    shared = {
        "wall": wall.astype(BF),
        "b16p": b16p.astype(BF),
    }
    bqc = np.asarray(inputs["bq"], np.float32).reshape(NRB, P).T
    bk8c = np.asarray(inputs["bk"], np.float32).reshape(NRB, P).T

    # f32 pack layout: selap(512) | cpack(CW) | w1e(128) | cp64(3) | bc8(512)
    F32W = 512 + CW + 128 + 3 + 128
    f32_base = np.zeros((P, F32W), np.float32)
    f32_base[:, 0:512] = _selector_const()
    f32_base[0:64, 512 + CW:512 + CW + 128] = w1e
    f32_base[0:64, 512 + CW + 128:512 + CW + 131] = \
        np.concatenate([acol, pcol_m, pcol_n], axis=1)
    f32_base[0:2, 512 + CW + 131:512 + CW + 259] = _bc8_const()

    in_maps = []
    for b in range(B):
        ko, qp = keyo[b], qo[b]
        G5, K5, Q5 = G5s[b], K5s[b], Q5s[b]

        xall = np.zeros((N + 2 * K5pad, D), BF)
        xall[0:N] = q[b][qp].astype(BF)
        xall[N:N + K5] = k[b][ko].astype(BF)
        xall[N + K5pad:N + K5pad + K5] = v[b][ko].astype(BF)

        cxk = np.zeros(GBLK * P, np.float32); cxk[:G5] = cx[b][ko[:G5]]
        cyk = np.zeros(GBLK * P, np.float32); cyk[:G5] = cy[b][ko[:G5]]
        l2wk = np.zeros(G5pad, np.float32); l2wk[:G5] = l2w[b][ko[:G5]]
        l2hk = np.zeros(G5pad, np.float32); l2hk[:G5] = l2h[b][ko[:G5]]
        bq4 = np.zeros((4, Q5pad), np.float32)
        bq4[2:] = 1.0
        l2q2 = np.zeros((2, Q5pad), np.float32)
        nq = min(Q5pad, N)
        bq4[0, :nq] = cx[b][qp[:nq]]; bq4[1, :nq] = cy[b][qp[:nq]]
        bq4[2, :nq] = 1.0 / ww[b][qp[:nq]]; bq4[3, :nq] = 1.0 / hh[b][qp[:nq]]
        l2q2[0, :nq] = l2w[b][qp[:nq]]; l2q2[1, :nq] = l2h[b][qp[:nq]]
        objq = np.zeros(Q5pad, np.float32)
        objq[:min(Q5, Q5pad)] = 1.0

        maskcol = np.full(NRB5 * P, -1e9 + ESHIFT, np.float32)
        maskcol[:K5] = ESHIFT
        maskcol = maskcol.reshape(NRB5, P).T

        bgm1 = np.zeros((P, n_geo), np.float32)
        epsm1 = np.zeros((P, n_geo), np.float32)
        for g in range(n_geo):
            for m in range(GM):
                key = g * GM + m
                for h in range(H):
                    if key < G5:
                        bgm1[h * GM + m, g] = bG[h] - 1.0
                        epsm1[h * GM + m, g] = 1e-6 - 1.0
                    else:
                        bgm1[h * GM + m, g] = -1e9
                        epsm1[h * GM + m, g] = 0.0

        f32p = f32_base.copy()
        f32p[:, 512:512 + CW] = np.concatenate([
            bqc, bk8c, maskcol,
            cxk.reshape(GBLK, P).T, cyk.reshape(GBLK, P).T,
            bgm1, epsm1], axis=1)

        l2kM = np.concatenate([np.broadcast_to(l2wk, (32, G5pad)),
                               np.broadcast_to(l2hk, (32, G5pad))], axis=0)
        l2qNh = np.concatenate([np.broadcast_to(l2q2[0], (32, Q5pad)),
                                np.broadcast_to(l2q2[1], (32, Q5pad))], axis=0)

        mm = dict(shared)
        mm.update({
            "xall": xall,
            "f32p": np.ascontiguousarray(f32p),
            "bq4": bq4,
            "l2kM": np.ascontiguousarray(l2kM),
            "l2qN": np.ascontiguousarray(l2qNh),
            "objq": objq.astype(BF),
        })
        in_maps.append(mm)

    inv_q = [np.argsort(qp) for qp in qo]
    return in_maps, sizes, inv_q


def build_nc(n_geo, GBLK, NRB5, Q5pad):
    K5pad = NRB5 * P
    G5pad = n_geo * GM
    XR = N + 2 * K5pad
    CW = 4 + 4 + NRB5 + GBLK + GBLK + n_geo + n_geo
    F32W = 512 + CW + 128 + 3 + 128
    nc = bass.Bass()

    def dp(name, shape, dt=F32):
        return nc.declare_dram_parameter(name, list(shape), dt, isOutput=False)

    XALL = dp("xall", (XR, D), BF16)
    WALL = dp("wall", (D, 4 * D), BF16)
    F32P = dp("f32p", (P, F32W))
    B16P = dp("b16p", (P, 1088), BF16)
    BQ4 = dp("bq4", (4, Q5pad))
    L2KM = dp("l2kM", (64, G5pad))
    L2QN = dp("l2qN", (64, Q5pad))
    OBJQ = dp("objq", (Q5pad,), BF16)
    out = nc.declare_dram_parameter("out", [N, D], F32, isOutput=True)
    wgd_dram = nc.dram_tensor("wgd_scratch", [n_geo, GM, H, Q5pad], BF16)

    with ExitStack() as ctx:
        tc = ctx.enter_context(tile.TileContext(nc))
        const = ctx.enter_context(tc.tile_pool(name="const", bufs=1))
        persist = ctx.enter_context(tc.tile_pool(name="persist", bufs=1))

        # ------------- constants (ACT queue) --------------------------------
        f32p = const.tile([P, F32W], F32, tag="f32p")
        nc.scalar.dma_start(f32p[:], F32P[:])
        CPo = 512
        W1o = 512 + CW
        C64o = W1o + 128
        BC8o = C64o + 3
        bq_t = f32p[:, CPo:CPo + 4]
        bk8x_t = f32p[:, CPo + 4:CPo + 8]
        mcol_t = f32p[:, CPo + 8:CPo + 8 + NRB5]
        cxk_t = f32p[:, CPo + 8 + NRB5:CPo + 8 + NRB5 + GBLK]
        cyk_t = f32p[:, CPo + 8 + NRB5 + GBLK:CPo + 8 + NRB5 + 2 * GBLK]
        bg_o = CPo + 8 + NRB5 + 2 * GBLK
        bgm1_t = f32p[:, bg_o:bg_o + n_geo]
        epsm1_t = f32p[:, bg_o + n_geo:bg_o + 2 * n_geo]
        w1e_f = f32p[0:64, W1o:W1o + 128]
        acol_t = f32p[0:64, C64o:C64o + 1]
        pcolm_t = f32p[0:64, C64o + 1:C64o + 2]
        pcoln_t = f32p[0:64, C64o + 2:C64o + 3]

        def selap(r0, r1, q4):
            return f32p[r0:r1, q4 * P:(q4 + 1) * P]

        bc8_t = f32p[0:2, BC8o:BC8o + P]

        b16p = const.tile([P, 1088], BF16, tag="b16p")
        nc.scalar.dma_start(b16p[:], B16P[:])

        def oh8(h):
            return b16p[:, h * 8:(h + 1) * 8]

        def wblk(c):
            return b16p[:, 64 + c * P:64 + (c + 1) * P]
        bvrow = b16p[0:1, 576:1088]
        borow = b16p[32:33, 576:1088]

        bq4bc = const.tile([P, 4, Q5pad], F32, tag="bq4bc")
        nc.scalar.dma_start(bq4bc[:],
                            BQ4[None, :, :].to_broadcast((P, 4, Q5pad)))
        cxqbc = bq4bc[:, 0, :]; cyqbc = bq4bc[:, 1, :]
        iwqbc = bq4bc[:, 2, :]; ihqbc = bq4bc[:, 3, :]
        objqbc = const.tile([P, Q5pad], BF16, tag="objqbc")
        nc.scalar.dma_start(objqbc[:], OBJQ[None, :].to_broadcast((P, Q5pad)))
        l2kM = const.tile([64, G5pad], F32, tag="l2kM")
        nc.scalar.dma_start(l2kM[:], L2KM[:])
        l2qN = const.tile([64, Q5pad], F32, tag="l2qN")
        nc.scalar.dma_start(l2qN[:], L2QN[:])
        halfpi_t = const.tile([P, 1], F32, tag="halfpi")
        nc.vector.memset(halfpi_t[:], HALF_PI)
        oh2 = const.tile([P, 4], BF16, tag="oh2")
        nc.vector.memset(oh2[:], 0.0)
        nc.vector.memset(oh2[:, 0:1], 1.0)
        nc.vector.memset(oh2[:, 3:4], 1.0)
        ones33_bf = const.tile([33, P], BF16, tag="ones33")
        nc.vector.memset(ones33_bf[:], 1.0)

        # ------------- input loads first (PE critical path, SP queue) -------
        xallT = persist.tile([P, NRB, XR], BF16, tag="xallT")
        wall = persist.tile([P, NRB, 4 * D], BF16, tag="wall")
        for cb in range(NRB):
            nc.sync.dma_start_transpose(xallT[:, cb, :],
                                        XALL[:, cb * P:(cb + 1) * P])
        nc.sync.dma_start(wall[:], WALL.rearrange("(kb p) d -> p kb d", p=P))

        def xqT(kb):
            return xallT[:, kb, 0:N]

        def xkT(kb):
            return xallT[:, kb, N:N + K5pad]

        def xvT(kb, c0, c1):
            return xallT[:, kb, N + K5pad + c0:N + K5pad + c1]

        def wsl(wi, kb, c0, c1):
            return wall[:, kb, wi * D + c0:wi * D + c1]

        # ---------------- phase 2: ln fields (pipelined) --------------------
        qT = persist.tile([P, NRB, N], BF16, tag="qT")
        kTt = persist.tile([P, NRB, K5pad], BF16, tag="kT")
        v_sb = persist.tile([P, NRB5, D], BF16, tag="v_sb")
        dxy2 = persist.tile([P, GBLK, 2, Q5pad], F32, tag="dxy2")
        with tc.tile_pool(name="work2", bufs=5) as work2:
            items = [(blk, ci) for blk in range(GBLK) for ci in range(2)]
            dws = {}
            l2s = {}
            for (blk, ci) in items:
                cbc = cxqbc if ci == 0 else cyqbc
                ccol = cxk_t if ci == 0 else cyk_t
                ibc = iwqbc if ci == 0 else ihqbc
                d_ = work2.tile([P, Q5pad], F32, tag="geo_d")
                nc.vector.tensor_scalar(d_[:], cbc, ccol[:, blk:blk + 1],
                                        None, ALU.subtract)
                dw_ = work2.tile([P, Q5pad], F32, tag="geo_dw")
                nc.vector.tensor_tensor(dw_[:], d_[:], ibc, ALU.mult)
                dws[(blk, ci)] = dw_
            for (blk, ci) in items:
                d2 = work2.tile([P, Q5pad], F32, tag="geo_d2")
                nc.scalar.activation(d2[:], dws[(blk, ci)][:], AF.Square)
                l2t = work2.tile([P, Q5pad], F32, tag="geo_l2")
                nc.scalar.activation(l2t[:], d2[:], AF.Ln)
                l2s[(blk, ci)] = l2t
            for (blk, ci) in items:
                nc.vector.tensor_scalar_max(dxy2[:, blk, ci, :],
                                            l2s[(blk, ci)][:], C2)

        # ---------------- phase 3: dw/dh banks ----------------
        bankM = persist.tile([64, G5pad], BF16, tag="bankM")
        bankN = persist.tile([64, Q5pad], BF16, tag="bankN")
        with tc.tile_pool(name="work3", bufs=2) as work3:
            bk_items = ((pcolm_t, l2kM, G5pad, bankM),
                        (pcoln_t, l2qN, Q5pad, bankN))
            fs = []
            for (pcol, l2bc, width, bank) in bk_items:
                t_ = work3.tile([64, width], F32, tag="bk_t")
                nc.vector.tensor_scalar(t_[:], l2bc[:], acol_t, pcol,
                                        ALU.mult, ALU.add)
                r_ = work3.tile([64, width], F32, tag="bk_r")
                nc.vector.tensor_scalar(r_[:], t_[:], MAGIC, -MAGIC,
                                        ALU.add, ALU.add)
                f_ = work3.tile([64, width], F32, tag="bk_f")
                nc.vector.tensor_tensor(f_[:], t_[:], r_[:], ALU.subtract)
                fs.append(f_)
            for (f_, (pcol, l2bc, width, bank)) in zip(fs, bk_items):
                nc.scalar.activation(bank[:], f_[:], AF.Sin, scale=TWO_PI)

        # ------- phase 4: geometry weights + interleaved projections --------
        # proj groups: (kind, idx); evictions on ACT so DVE stays on folds
        groups = ([("q", ob) for ob in range(NRB)]
                  + [("k", ob) for ob in range(NRB)]
                  + [("v", mb) for mb in range(NRB5)])
        NGRP = len(groups)

        wgdT = persist.tile([P, GBLK, H, Q5pad], BF16, tag="wgdT")
        with tc.tile_pool(name="work4", bufs=3) as work4, \
             tc.tile_pool(name="psum_u", bufs=2, space="PSUM") as psum_u, \
             tc.tile_pool(name="psum_p", bufs=2, space="PSUM") as psum_p, \
             tc.tile_pool(name="psum_wg", bufs=2, space="PSUM") as psum_wg:
            wgps = [None] * n_geo
            gps = [None] * NGRP

            def emit_group(j):
                kind, ob = groups[j]
                ps = psum_p.tile([P, N], F32, tag="pps")
                if kind == "q":
                    for kb in range(NRB):
                        nc.tensor.matmul(ps[:],
                                         wsl(0, kb, ob * P, (ob + 1) * P),
                                         xqT(kb),
                                         start=(kb == 0),
                                         stop=(kb == NRB - 1))
                elif kind == "k":
                    for kb in range(NRB):
                        nc.tensor.matmul(ps[:, :K5pad],
                                         wsl(1, kb, ob * P, (ob + 1) * P),
                                         xkT(kb),
                                         start=(kb == 0),
                                         stop=(kb == NRB - 1))
                else:
                    for kb in range(NRB):
                        nc.tensor.matmul(ps[:], xvT(kb, ob * P, (ob + 1) * P),
                                         wsl(2, kb, 0, D),
                                         start=(kb == 0), stop=False)
                    nc.tensor.matmul(ps[:], ones33_bf[0:1, :], bvrow,
                                     start=False, stop=True)
                gps[j] = ps

            def evict_group(j):
                kind, ob = groups[j]
                ps = gps[j]
                if kind == "q":
                    nc.scalar.activation(qT[:, ob, :], ps[:], AF.Identity,
                                         bias=bq_t[:, ob:ob + 1])
                elif kind == "k":
                    nc.scalar.activation(kTt[:, ob, :], ps[:, :K5pad],
                                         AF.Identity, scale=0.125,
                                         bias=bk8x_t[:, ob:ob + 1])
                else:
                    nc.scalar.copy(v_sb[:, ob, :], ps[:])

            # distribute groups over geo iters (emit_group(j) at iter sched[j])
            sched = {}
            for j in range(NGRP):
                sched.setdefault(min(j * n_geo // NGRP, n_geo - 1), []).append(j)

            def stage_b(g):
                wgdB = work4.tile([P, Q5pad], BF16, tag="wgdB")
                nc.vector.tensor_scalar(wgdB[:], wgps[g][:, :Q5pad],
                                        bgm1_t[:, g:g + 1],
                                        epsm1_t[:, g:g + 1],
                                        ALU.add, ALU.max)
                wgdm1 = work4.tile([P, Q5pad], BF16, tag="wgdm1")
                nc.gpsimd.tensor_tensor(wgdm1[:], wgdB[:], objqbc[:],
                                        ALU.mult)
                wgdM = work4.tile([P, Q5pad], BF16, tag="wgdM")
                nc.gpsimd.tensor_scalar(wgdM[:], wgdm1[:], 1.0, None, ALU.add)
                nc.scalar.dma_start(
                    wgd_dram[g].rearrange("t h q -> h t q"), wgdM[:])

            prev_groups = []
            for g in range(n_geo):
                blk = g // 8
                off = 64 * ((g % 8) // 4)
                q4 = g % 4
                mbase = g * GM
                lhs_wh = work4.tile([64, P], BF16, tag="lhs_wh")
                nc.vector.tensor_tensor(
                    lhs_wh[:].rearrange("k (h m) -> k h m", h=H),
                    w1e_f.rearrange("k (h m) -> k h m", h=H),
                    bankM[:, mbase:mbase + GM][:, None, :]
                        .to_broadcast((64, H, GM)),
                    ALU.mult)
                ups = psum_u.tile([P, 2, N], F32, tag="ups")
                for ci in range(2):
                    nc.tensor.matmul(ups[:, ci, :Q5pad],
                                     selap(off, off + 64, q4),
                                     dxy2[off:off + 64, blk, ci, :],
                                     start=True, stop=True)
                # projections fill the PE gap while DVE folds
                for j in sched.get(g, []):
                    emit_group(j)
                upsv = ups[:, :, :Q5pad]
                rrS = work4.tile([P, 2, Q5pad], F32, tag="rrS")
                nc.vector.tensor_scalar(rrS[:], upsv, MAGIC, -MAGIC,
                                        ALU.add, ALU.add)
                nfS = work4.tile([P, 2, Q5pad], F32, tag="nfS")
                nc.vector.tensor_tensor(nfS[:], rrS[:], upsv, ALU.subtract)
                # cos fold from nfS: nfC = nfS + (nfS <= -0.25)
                ind = work4.tile([P, 2, Q5pad], F32, tag="ind")
                nc.vector.tensor_scalar(ind[:], nfS[:], -0.25, None,
                                        ALU.is_le)
                nfC = work4.tile([P, 2, Q5pad], F32, tag="nfC")
                nc.vector.tensor_tensor(nfC[:], nfS[:], ind[:], ALU.add)
                sS = work4.tile([P, 2, Q5pad], BF16, tag="sS")
                nc.scalar.activation(sS[:], nfS[:], AF.Sin, scale=-TWO_PI)
                sC = work4.tile([P, 2, Q5pad], BF16, tag="sC")
                nc.scalar.activation(sC[:], nfC[:], AF.Sin, scale=-TWO_PI,
                                     bias=halfpi_t[:])
                wgp = psum_wg.tile([P, N], F32, tag="wgp")
                nc.tensor.matmul(wgp[:, :Q5pad], wblk(0), sS[:, 0, :],
                                 start=True, stop=False)
                nc.tensor.matmul(wgp[:, :Q5pad], wblk(1), sC[:, 0, :],
                                 start=False, stop=False)
                nc.tensor.matmul(wgp[:, :Q5pad], wblk(2), sS[:, 1, :],
                                 start=False, stop=False)
                nc.tensor.matmul(wgp[:, :Q5pad], wblk(3), sC[:, 1, :],
                                 start=False, stop=False)
                nc.tensor.matmul(wgp[:, :Q5pad], lhs_wh[:], bankN[:],
                                 start=False, stop=True)
                wgps[g] = wgp
                if g >= 1:
                    stage_b(g - 1)
                for j in prev_groups:
                    evict_group(j)
                prev_groups = sched.get(g, [])
            stage_b(n_geo - 1)
            for j in prev_groups:
                evict_group(j)
            # gather to attention layout: one DMA per block
            for blk in range(GBLK):
                gcnt = min(8, n_geo - blk * 8)
                nc.scalar.dma_start(
                    wgdT[0:gcnt * GM, blk, :, :],
                    wgd_dram[blk * 8:blk * 8 + gcnt]
                        .rearrange("g t h q -> (g t) h q"))

        # ---------------- phase 5: attention (2-stage pipeline) -------------
        ot = persist.tile([P, NRB, N], BF16, tag="ot")
        with tc.tile_pool(name="work5", bufs=4) as work5, \
             tc.tile_pool(name="psum5", bufs=2, space="PSUM") as psum5, \
             tc.tile_pool(name="psum_s", bufs=1, space="PSUM") as psum_s, \
             tc.tile_pool(name="psum_r", bufs=1, space="PSUM") as psum_r, \
             tc.tile_pool(name="psum_av", bufs=1, space="PSUM") as psum_av:
            flat = [(ob, rb) for ob in range(NRB) for rb in range(NRB5)]
            avs, sbanks, es = {}, {}, {}

            def accum(ob, rb):
                h0 = 2 * ob
                if rb == 0:
                    av_t = psum_av.tile([P, N], F32, tag="avps")
                    sb_t = psum_s.tile([2, N], F32, tag="sbank")
                    avs[ob] = av_t
                    sbanks[ob] = sb_t
                e_ = es[(ob, rb)]
                for hi in range(2):
                    po = hi * DK
                    nc.tensor.matmul(sbanks[ob][:], oh2[:, 2 * hi:2 * hi + 2],
                                     e_[:, hi, :],
                                     start=(rb == 0 and hi == 0),
                                     stop=(rb == NRB5 - 1 and hi == 1),
                                     skip_group_check=True)
                    nc.tensor.matmul(avs[ob][po:po + DK, :],
                                     v_sb[:, rb,
                                          (h0 + hi) * DK:(h0 + hi + 1) * DK],
                                     e_[:, hi, :], start=(rb == 0),
                                     stop=(rb == NRB5 - 1),
                                     skip_group_check=True)

            def finish(ob):
                rs = work5.tile([2, N], F32, tag="rs")
                nc.vector.reciprocal(rs[:], sbanks[ob][:])
                rrp = psum_r.tile([P, N], F32, tag="rrp")
                nc.tensor.matmul(rrp[:], bc8_t, rs[:], start=True, stop=True)
                rr_b = work5.tile([P, N], F32, tag="rr_b")
                nc.scalar.copy(rr_b[:], rrp[:])
                nc.vector.tensor_tensor(ot[:, ob, :], avs[ob][:], rr_b[:],
                                        ALU.mult)

            for i, (ob, rb) in enumerate(flat):
                h0 = 2 * ob
                st2 = psum5.tile([P, 2, N], F32, tag="stps")
                for hi in range(2):
                    po = hi * DK
                    nc.tensor.matmul(
                        st2[:, hi, :],
                        kTt[po:po + DK, ob, rb * P:(rb + 1) * P],
                        qT[po:po + DK, ob, :], start=True, stop=True)
                e_ = work5.tile([P, 2, N], BF16, tag="e_t")
                nc.scalar.activation(e_[:], st2[:], AF.Exp,
                                     bias=mcol_t[:, rb:rb + 1])
                if rb < GBLK:
                    rows = min(P, G5pad - rb * P)
                    nc.vector.tensor_tensor(
                        e_[0:rows, :, 0:Q5pad], e_[0:rows, :, 0:Q5pad],
                        wgdT[0:rows, rb, h0:h0 + 2, :], ALU.mult)
                es[(ob, rb)] = e_
                if i >= 1:
                    accum(*flat[i - 1])
                    if flat[i - 1][1] == NRB5 - 1:
                        finish(flat[i - 1][0])
            accum(*flat[-1])
            finish(flat[-1][0])

        # ---------------- phase 6: output projection ----------------
        with tc.tile_pool(name="work6", bufs=2) as work6, \
             tc.tile_pool(name="psum6", bufs=2, space="PSUM") as psum6:
            fps = []

            def evict(rr):
                fo = work6.tile([P, D], F32, tag="fo")
                if rr % 2 == 0:
                    nc.scalar.copy(fo[:], fps[rr][:])
                else:
                    nc.vector.tensor_copy(fo[:], fps[rr][:])
                nc.sync.dma_start(out[rr * P:(rr + 1) * P, :], fo[:])

            for r in range(NRB):
                ps = psum6.tile([P, D], F32, tag="fps")
                for kt in range(NRB):
                    nc.tensor.matmul(ps[:], ot[:, kt, r * P:(r + 1) * P],
                                     wsl(3, kt, 0, D),
                                     start=(kt == 0), stop=False)
                nc.tensor.matmul(ps[:], ones33_bf[32:33, :], borow,
                                 start=False, stop=True)
                fps.append(ps)
                if r >= 1:
                    evict(r - 1)
            evict(NRB - 1)

    _split_multi_waits(nc)
    return nc


_NC_CACHE = {}


def kernel(**inputs):
    in_maps, sizes, inv_q = _host_prep(inputs)
    if _NC_CACHE.get("sizes") != sizes:
        _NC_CACHE["nc"] = build_nc(*sizes)
        _NC_CACHE["sizes"] = sizes
    nc = _NC_CACHE["nc"]
    res = run_bass_kernel_spmd(nc, in_maps, list(range(B)))
    out = np.stack([res.results[b]["out"][inv_q[b]] for b in range(B)], axis=0)
    return out.astype(np.float32)


if __name__ == "__main__":
    print("kernel module ok")


# revision 20
# speedup vs baseline: 1.0230x; 1.0002x over previous
"""Trainium2 Bass kernel for BoxMultiHeadedAttention (B=8, N=512, D=512, H=8).

Sharding: data-parallel over batch — each of the 8 NeuronCores computes one
batch element end-to-end; weights replicated; no collectives.

Sparsity compaction (host-side, per call; sizes padded to the max over the
8 batch elements so a single SPMD program serves all cores):
  * keys with mask==0 contribute exp(-1e9)=0 -> dropped entirely;
    kept keys ordered [mask&obj ("geo" keys) | mask&~obj], padded to
    NRB5*128 with -1e9 mask columns.
  * queries permuted obj-first: the geometry bias only applies to
    (obj_i & obj_j) pairs, so wg is computed for geo-keys x obj-queries
    only; per-core residual regions are neutralized with data
    ([P,1] bias/clip vectors and an obj-query column mask).
  * output rows are inverse-permuted on the host.

Per-core algorithm (layout [keys(part), queries(free)] throughout):
  * x shipped bf16 as one concatenated [xq;xk;xv] matrix -> 4 XBAR
    DMA-transposes; all weights in one packed DMA; all f32/bf16 consts in
    one packed DMA each (DMA issue is latency-chained, so count matters).
  * geometry: g = clip(ln((dx/w_i)^2), C2) on DVE+ACT; phases t = a/(4pi)*g
    via f32 selector matmul; sin/cos by exact magic-number folds
    (sin(2pi t) = Sin(-2pi*(round(t)-t)); cos via round(t+1/4) and
    bias pi/2); per-head contraction on PE (bf16); dw/dh separable
    rank-64 bank contraction.
  * wg multiplier M = 1 + max(wg+bG-1, 1e-6-1)*objq masked per-core via
    [P,1] vectors; routed to attention layout through a DRAM roundtrip
    (plain-SBUF DMAs; the (h,g) permutation lives in DRAM-side APs).
  * exp-domain softmax: T = E * M on the geo sub-tile only; row sums via
    ones-matmul; 1/rowsum broadcast across partitions by an exact f32
    selection matmul (no DRAM roundtrip); output projection bias folded
    in as a ones-row matmul.
All loops are software-pipelined (finalization of iter g emitted after the
start of iter g+1) so the in-order engine queues never head-of-line block.
"""
import math
import numpy as np
import ml_dtypes
from contextlib import ExitStack

import concourse.bass as bass
import concourse.mybir as mybir
import concourse.tile as tile
from concourse.bass_utils import run_bass_kernel_spmd

F32 = mybir.dt.float32
BF16 = mybir.dt.bfloat16
AF = mybir.ActivationFunctionType
ALU = mybir.AluOpType

B, N, D, H = 8, 512, 512, 8
DK = D // H
P = 128
NRB = N // P
GM = 16
WAVE_LEN = 1000.0
MAGIC = 12582912.0
C2 = float(2.0 * math.log(0.001))
ESHIFT = -6.0
TWO_PI = float(2.0 * math.pi)
HALF_PI = float(math.pi / 2.0)

_alphas = (100.0 / (WAVE_LEN ** (np.arange(8) / 8.0))).astype(np.float64)
BF = ml_dtypes.bfloat16


def _split_multi_waits(nc):
    """walrus accepts only ONE sync-wait per ISA instruction; hoist extras
    onto NoOps inserted before the offending instruction."""
    n_fix = 0
    for blk in nc.main_func.blocks:
        insts = list(blk.instructions)
        out, dirty = [], False
        for inst in insts:
            si = inst.sync_info
            waits = list(si.on_wait) if si is not None else []
            if len(waits) > 1:
                for kk, w in enumerate(waits[:-1]):
                    out.append(mybir.InstNoOp(
                        name=f"I-waitfix-{n_fix}-{kk}", engine=inst.engine,
                        sync_info=mybir.SyncInfo(on_wait=[w], on_update=[])))
                inst.sync_info = mybir.SyncInfo(
                    on_wait=[waits[-1]], on_update=list(si.on_update))
                n_fix += 1
                dirty = True
            out.append(inst)
        if dirty:
            blk.instructions = out
    return n_fix


def _selector_const():
    # SELAP[64*W + q*16 + m_loc, q*128 + m_loc*8 + j] = alpha_j/(4pi)
    selap = np.zeros((P, 4, P), dtype=np.float32)
    for W in range(2):
        for q in range(4):
            for m_loc in range(GM):
                for j in range(8):
                    selap[64 * W + q * 16 + m_loc, q, m_loc * 8 + j] = \
                        _alphas[j] / (4.0 * math.pi)
    return selap.reshape(P, 4 * P)


def _onehot8():
    oh = np.zeros((P, H, H), dtype=np.float32)
    for h in range(H):
        oh[:, h, h] = 1.0
    return oh.reshape(P, H * H)


def _wblk_direct(WG):
    # direct sin/cos weights: c in (sin-x, cos-x, sin-y, cos-y)
    gmap = [lambda j: j, lambda j: 32 + j, lambda j: 8 + j, lambda j: 40 + j]
    wblk = np.zeros((P, 4, P), dtype=np.float32)
    for c in range(4):
        for m_loc in range(GM):
            for j in range(8):
                for h in range(H):
                    wblk[m_loc * 8 + j, c, h * GM + m_loc] = WG[h, gmap[c](j)]
    return wblk.reshape(P, 4 * P)


def _bank_consts(WG):
    # dw/dh rank-64 decomposition (sin(A-B) via quarter-phase shifts)
    acol = np.zeros((64, 1), np.float32)
    pcol_m = np.zeros((64, 1), np.float32)
    pcol_n = np.zeros((64, 1), np.float32)
    w1 = np.zeros((64, H), np.float32)
    for f in range(2):
        for j in range(8):
            gs = 16 + 8 * f + j
            gc = 48 + 8 * f + j
            a = _alphas[j] / (4.0 * math.pi)
            for t in range(4):
                k = (f * 8 + j) * 4 + t
                acol[k, 0] = a
                pcol_m[k, 0] = 0.25 if t in (0, 2) else 0.0
                if t == 0:
                    pcol_n[k, 0] = 0.0; w1[k] = WG[:, gs]
                elif t == 1:
                    pcol_n[k, 0] = 0.75; w1[k] = WG[:, gs]   # -cos -> +pi
                elif t == 2:
                    pcol_n[k, 0] = 0.25; w1[k] = WG[:, gc]
                else:
                    pcol_n[k, 0] = 0.0; w1[k] = WG[:, gc]
    w1e = np.repeat(w1, GM, axis=1).astype(np.float32)
    return acol, pcol_m, pcol_n, w1e


def _bc8_const():
    # rr_b[p, n] = rs2[p//64, n]: bc8[k, p] = 1 iff k == p//64
    bc8 = np.zeros((2, P), np.float32)
    for p_ in range(P):
        bc8[p_ // 64, p_] = 1.0
    return bc8


def _host_prep(inputs):
    q = np.asarray(inputs["input_query"], np.float32)
    k = np.asarray(inputs["input_key"], np.float32)
    v = np.asarray(inputs["input_value"], np.float32)
    box = np.asarray(inputs["input_box"], np.float32)
    mask = np.asarray(inputs["mask"])
    nobj = np.asarray(inputs["not_objects"])
    WG = np.asarray(inputs["WG"], np.float32)
    bG = np.asarray(inputs["bG"], np.float32)

    x_min, y_min, x_max, y_max = [box[..., i] for i in range(4)]
    cx = (x_min + x_max) * 0.5
    cy = (y_min + y_max) * 0.5
    ww = x_max - x_min + 1.0
    hh = y_max - y_min + 1.0
    l2w = (2.0 * np.log(ww)).astype(np.float32)
    l2h = (2.0 * np.log(hh)).astype(np.float32)

    keyo, qo, G5s, K5s, Q5s = [], [], [], [], []
    for b in range(B):
        m_b = mask[b] != 0
        o_b = ~nobj[b]
        geo = np.where(m_b & o_b)[0]
        oth = np.where(m_b & ~o_b)[0]
        keyo.append(np.concatenate([geo, oth]))
        qobj = np.where(o_b)[0]
        qrest = np.where(~o_b)[0]
        qo.append(np.concatenate([qobj, qrest]))
        G5s.append(len(geo)); K5s.append(len(geo) + len(oth))
        Q5s.append(len(qobj))

    G5max = max(max(G5s), 1)
    n_geo = (G5max + GM - 1) // GM
    G5pad = n_geo * GM
    GBLK = (G5pad + P - 1) // P
    K5max = max(max(K5s), 1)
    NRB5 = (K5max + P - 1) // P
    K5pad = NRB5 * P
    Q5max = max(max(Q5s), 1)
    Q5pad = min(N, ((Q5max + 31) // 32) * 32)
    sizes = (n_geo, GBLK, NRB5, Q5pad)
    CW = 4 + 4 + NRB5 + GBLK + GBLK + n_geo + n_geo

    acol, pcol_m, pcol_n, w1e = _bank_consts(WG)
    # shared bf16 pack: oh8(64) | wblk(512) | bvbo(512)
    b16p = np.zeros((P, 64 + 512 + 512), np.float32)
    b16p[:, 0:64] = _onehot8()
    b16p[:, 64:576] = _wblk_direct(WG)
    b16p[0, 576:1088] = np.asarray(inputs["bv"], np.float32)
    b16p[32, 576:1088] = np.asarray(inputs["bo"], np.float32)
    wall = np.concatenate([
        np.asarray(inputs["Wq"], np.float32),
        np.asarray(inputs["Wk"], np.float32),
        np.asarray(inputs["Wv"], np.float32),
        np.asarray(inputs["Wo"], np.float32)], axis=1)  # [512, 2048]
    shared = {
        "wall": wall.astype(BF),
        "b16p": b16p.astype(BF),
    }
    bqc = np.asarray(inputs["bq"], np.float32).reshape(NRB, P).T
    bk8c = np.asarray(inputs["bk"], np.float32).reshape(NRB, P).T

    # f32 pack layout: selap(512) | cpack(CW) | w1e(128) | cp64(3) | bc8(512)
    F32W = 512 + CW + 128 + 3 + 128
    f32_base = np.zeros((P, F32W), np.float32)
    f32_base[:, 0:512] = _selector_const()
    f32_base[0:64, 512 + CW:512 + CW + 128] = w1e
    f32_base[0:64, 512 + CW + 128:512 + CW + 131] = \
        np.concatenate([acol, pcol_m, pcol_n], axis=1)
    f32_base[0:2, 512 + CW + 131:512 + CW + 259] = _bc8_const()

    in_maps = []
    for b in range(B):
        ko, qp = keyo[b], qo[b]
        G5, K5, Q5 = G5s[b], K5s[b], Q5s[b]

        xall = np.zeros((N + 2 * K5pad, D), BF)
        xall[0:N] = q[b][qp].astype(BF)
        xall[N:N + K5] = k[b][ko].astype(BF)
        xall[N + K5pad:N + K5pad + K5] = v[b][ko].astype(BF)

        cxk = np.zeros(GBLK * P, np.float32); cxk[:G5] = cx[b][ko[:G5]]
        cyk = np.zeros(GBLK * P, np.float32); cyk[:G5] = cy[b][ko[:G5]]
        l2wk = np.zeros(G5pad, np.float32); l2wk[:G5] = l2w[b][ko[:G5]]
        l2hk = np.zeros(G5pad, np.float32); l2hk[:G5] = l2h[b][ko[:G5]]
        bq4 = np.zeros((4, Q5pad), np.float32)
        bq4[2:] = 1.0
        l2q2 = np.zeros((2, Q5pad), np.float32)
        nq = min(Q5pad, N)
        bq4[0, :nq] = cx[b][qp[:nq]]; bq4[1, :nq] = cy[b][qp[:nq]]
        bq4[2, :nq] = 1.0 / ww[b][qp[:nq]]; bq4[3, :nq] = 1.0 / hh[b][qp[:nq]]
        l2q2[0, :nq] = l2w[b][qp[:nq]]; l2q2[1, :nq] = l2h[b][qp[:nq]]
        objq = np.zeros(Q5pad, np.float32)
        objq[:min(Q5, Q5pad)] = 1.0

        maskcol = np.full(NRB5 * P, -1e9 + ESHIFT, np.float32)
        maskcol[:K5] = ESHIFT
        maskcol = maskcol.reshape(NRB5, P).T

        bgm1 = np.zeros((P, n_geo), np.float32)
        epsm1 = np.zeros((P, n_geo), np.float32)
        for g in range(n_geo):
            for m in range(GM):
                key = g * GM + m
                for h in range(H):
                    if key < G5:
                        bgm1[h * GM + m, g] = bG[h] - 1.0
                        epsm1[h * GM + m, g] = 1e-6 - 1.0
                    else:
                        bgm1[h * GM + m, g] = -1e9
                        epsm1[h * GM + m, g] = 0.0

        f32p = f32_base.copy()
        f32p[:, 512:512 + CW] = np.concatenate([
            bqc, bk8c, maskcol,
            cxk.reshape(GBLK, P).T, cyk.reshape(GBLK, P).T,
            bgm1, epsm1], axis=1)

        l2kM = np.concatenate([np.broadcast_to(l2wk, (32, G5pad)),
                               np.broadcast_to(l2hk, (32, G5pad))], axis=0)
        l2qNh = np.concatenate([np.broadcast_to(l2q2[0], (32, Q5pad)),
                                np.broadcast_to(l2q2[1], (32, Q5pad))], axis=0)

        mm = dict(shared)
        mm.update({
            "xall": xall,
            "f32p": np.ascontiguousarray(f32p),
            "bq4": bq4,
            "l2kM": np.ascontiguousarray(l2kM),
            "l2qN": np.ascontiguousarray(l2qNh),
            "objq": objq.astype(BF),
        })
        in_maps.append(mm)

    inv_q = [np.argsort(qp) for qp in qo]
    return in_maps, sizes, inv_q


def build_nc(n_geo, GBLK, NRB5, Q5pad):
    K5pad = NRB5 * P
    G5pad = n_geo * GM
    XR = N + 2 * K5pad
    CW = 4 + 4 + NRB5 + GBLK + GBLK + n_geo + n_geo
    F32W = 512 + CW + 128 + 3 + 128
    nc = bass.Bass()

    def dp(name, shape, dt=F32):
        return nc.declare_dram_parameter(name, list(shape), dt, isOutput=False)

    XALL = dp("xall", (XR, D), BF16)
    WALL = dp("wall", (D, 4 * D), BF16)
    F32P = dp("f32p", (P, F32W))
    B16P = dp("b16p", (P, 1088), BF16)
    BQ4 = dp("bq4", (4, Q5pad))
    L2KM = dp("l2kM", (64, G5pad))
    L2QN = dp("l2qN", (64, Q5pad))
    OBJQ = dp("objq", (Q5pad,), BF16)
    out = nc.declare_dram_parameter("out", [N, D], F32, isOutput=True)
    wgd_dram = nc.dram_tensor("wgd_scratch", [n_geo, GM, H, Q5pad], BF16)

    with ExitStack() as ctx:
        tc = ctx.enter_context(tile.TileContext(nc))
        const = ctx.enter_context(tc.tile_pool(name="const", bufs=1))
        persist = ctx.enter_context(tc.tile_pool(name="persist", bufs=1))

        # ------------- constants (ACT queue) --------------------------------
        f32p = const.tile([P, F32W], F32, tag="f32p")
        nc.scalar.dma_start(f32p[:], F32P[:])
        CPo = 512
        W1o = 512 + CW
        C64o = W1o + 128
        BC8o = C64o + 3
        bq_t = f32p[:, CPo:CPo + 4]
        bk8x_t = f32p[:, CPo + 4:CPo + 8]
        mcol_t = f32p[:, CPo + 8:CPo + 8 + NRB5]
        cxk_t = f32p[:, CPo + 8 + NRB5:CPo + 8 + NRB5 + GBLK]
        cyk_t = f32p[:, CPo + 8 + NRB5 + GBLK:CPo + 8 + NRB5 + 2 * GBLK]
        bg_o = CPo + 8 + NRB5 + 2 * GBLK
        bgm1_t = f32p[:, bg_o:bg_o + n_geo]
        epsm1_t = f32p[:, bg_o + n_geo:bg_o + 2 * n_geo]
        w1e_f = f32p[0:64, W1o:W1o + 128]
        acol_t = f32p[0:64, C64o:C64o + 1]
        pcolm_t = f32p[0:64, C64o + 1:C64o + 2]
        pcoln_t = f32p[0:64, C64o + 2:C64o + 3]

        def selap(r0, r1, q4):
            return f32p[r0:r1, q4 * P:(q4 + 1) * P]

        bc8_t = f32p[0:2, BC8o:BC8o + P]

        b16p = const.tile([P, 1088], BF16, tag="b16p")
        nc.scalar.dma_start(b16p[:], B16P[:])

        def oh8(h):
            return b16p[:, h * 8:(h + 1) * 8]

        def wblk(c):
            return b16p[:, 64 + c * P:64 + (c + 1) * P]
        bvrow = b16p[0:1, 576:1088]
        borow = b16p[32:33, 576:1088]

        bq4bc = const.tile([P, 4, Q5pad], F32, tag="bq4bc")
        nc.scalar.dma_start(bq4bc[:],
                            BQ4[None, :, :].to_broadcast((P, 4, Q5pad)))
        cxqbc = bq4bc[:, 0, :]; cyqbc = bq4bc[:, 1, :]
        iwqbc = bq4bc[:, 2, :]; ihqbc = bq4bc[:, 3, :]
        objqbc = const.tile([P, Q5pad], BF16, tag="objqbc")
        nc.scalar.dma_start(objqbc[:], OBJQ[None, :].to_broadcast((P, Q5pad)))
        l2kM = const.tile([64, G5pad], F32, tag="l2kM")
        nc.scalar.dma_start(l2kM[:], L2KM[:])
        l2qN = const.tile([64, Q5pad], F32, tag="l2qN")
        nc.scalar.dma_start(l2qN[:], L2QN[:])
        halfpi_t = const.tile([P, 1], F32, tag="halfpi")
        nc.vector.memset(halfpi_t[:], HALF_PI)
        oh2 = const.tile([P, 4], BF16, tag="oh2")
        nc.vector.memset(oh2[:], 0.0)
        nc.vector.memset(oh2[:, 0:1], 1.0)
        nc.vector.memset(oh2[:, 3:4], 1.0)
        ones33_bf = const.tile([33, P], BF16, tag="ones33")
        nc.vector.memset(ones33_bf[:], 1.0)

        # ------------- input loads first (PE critical path, SP queue) -------
        xallT = persist.tile([P, NRB, XR], BF16, tag="xallT")
        wall = persist.tile([P, NRB, 4 * D], BF16, tag="wall")
        for cb in range(NRB):
            nc.sync.dma_start_transpose(xallT[:, cb, :],
                                        XALL[:, cb * P:(cb + 1) * P])
        nc.sync.dma_start(wall[:], WALL.rearrange("(kb p) d -> p kb d", p=P))

        def xqT(kb):
            return xallT[:, kb, 0:N]

        def xkT(kb):
            return xallT[:, kb, N:N + K5pad]

        def xvT(kb, c0, c1):
            return xallT[:, kb, N + K5pad + c0:N + K5pad + c1]

        def wsl(wi, kb, c0, c1):
            return wall[:, kb, wi * D + c0:wi * D + c1]

        # ---------------- phase 2: ln fields (pipelined) --------------------
        qT = persist.tile([P, NRB, N], BF16, tag="qT")
        kTt = persist.tile([P, NRB, K5pad], BF16, tag="kT")
        v_sb = persist.tile([P, NRB5, D], BF16, tag="v_sb")
        dxy2 = persist.tile([P, GBLK, 2, Q5pad], F32, tag="dxy2")
        with tc.tile_pool(name="work2", bufs=5) as work2:
            items = [(blk, ci) for blk in range(GBLK) for ci in range(2)]
            dws = {}
            l2s = {}
            for (blk, ci) in items:
                cbc = cxqbc if ci == 0 else cyqbc
                ccol = cxk_t if ci == 0 else cyk_t
                ibc = iwqbc if ci == 0 else ihqbc
                d_ = work2.tile([P, Q5pad], F32, tag="geo_d")
                nc.vector.tensor_scalar(d_[:], cbc, ccol[:, blk:blk + 1],
                                        None, ALU.subtract)
                dw_ = work2.tile([P, Q5pad], F32, tag="geo_dw")
                nc.vector.tensor_tensor(dw_[:], d_[:], ibc, ALU.mult)
                dws[(blk, ci)] = dw_
            for (blk, ci) in items:
                d2 = work2.tile([P, Q5pad], F32, tag="geo_d2")
                nc.scalar.activation(d2[:], dws[(blk, ci)][:], AF.Square)
                l2t = work2.tile([P, Q5pad], F32, tag="geo_l2")
                nc.scalar.activation(l2t[:], d2[:], AF.Ln)
                l2s[(blk, ci)] = l2t
            for (blk, ci) in items:
                nc.vector.tensor_scalar_max(dxy2[:, blk, ci, :],
                                            l2s[(blk, ci)][:], C2)

        # ---------------- phase 3: dw/dh banks ----------------
        bankM = persist.tile([64, G5pad], BF16, tag="bankM")
        bankN = persist.tile([64, Q5pad], BF16, tag="bankN")
        with tc.tile_pool(name="work3", bufs=2) as work3:
            bk_items = ((pcolm_t, l2kM, G5pad, bankM),
                        (pcoln_t, l2qN, Q5pad, bankN))
            fs = []
            for (pcol, l2bc, width, bank) in bk_items:
                t_ = work3.tile([64, width], F32, tag="bk_t")
                nc.vector.tensor_scalar(t_[:], l2bc[:], acol_t, pcol,
                                        ALU.mult, ALU.add)
                r_ = work3.tile([64, width], F32, tag="bk_r")
                nc.vector.tensor_scalar(r_[:], t_[:], MAGIC, -MAGIC,
                                        ALU.add, ALU.add)
                f_ = work3.tile([64, width], F32, tag="bk_f")
                nc.vector.tensor_tensor(f_[:], t_[:], r_[:], ALU.subtract)
                fs.append(f_)
            for (f_, (pcol, l2bc, width, bank)) in zip(fs, bk_items):
                nc.scalar.activation(bank[:], f_[:], AF.Sin, scale=TWO_PI)

        # ------- phase 4: geometry weights + interleaved projections --------
        # proj groups: (kind, idx); evictions on ACT so DVE stays on folds
        groups = ([("q", ob) for ob in range(NRB)]
                  + [("k", ob) for ob in range(NRB)]
                  + [("v", mb) for mb in range(NRB5)])
        NGRP = len(groups)

        wgdT = persist.tile([P, GBLK, H, Q5pad], BF16, tag="wgdT")
        with tc.tile_pool(name="work4", bufs=3) as work4, \
             tc.tile_pool(name="psum_u", bufs=2, space="PSUM") as psum_u, \
             tc.tile_pool(name="psum_p", bufs=2, space="PSUM") as psum_p, \
             tc.tile_pool(name="psum_wg", bufs=2, space="PSUM") as psum_wg:
            wgps = [None] * n_geo
            gps = [None] * NGRP

            def emit_group(j):
                kind, ob = groups[j]
                ps = psum_p.tile([P, N], F32, tag="pps")
                if kind == "q":
                    for kb in range(NRB):
                        nc.tensor.matmul(ps[:],
                                         wsl(0, kb, ob * P, (ob + 1) * P),
                                         xqT(kb),
                                         start=(kb == 0),
                                         stop=(kb == NRB - 1))
                elif kind == "k":
                    for kb in range(NRB):
                        nc.tensor.matmul(ps[:, :K5pad],
                                         wsl(1, kb, ob * P, (ob + 1) * P),
                                         xkT(kb),
                                         start=(kb == 0),
                                         stop=(kb == NRB - 1))
                else:
                    for kb in range(NRB):
                        nc.tensor.matmul(ps[:], xvT(kb, ob * P, (ob + 1) * P),
                                         wsl(2, kb, 0, D),
                                         start=(kb == 0), stop=False)
                    nc.tensor.matmul(ps[:], ones33_bf[0:1, :], bvrow,
                                     start=False, stop=True)
                gps[j] = ps

            def evict_group(j):
                kind, ob = groups[j]
                ps = gps[j]
                if kind == "q":
                    nc.scalar.activation(qT[:, ob, :], ps[:], AF.Identity,
                                         bias=bq_t[:, ob:ob + 1])
                elif kind == "k":
                    nc.scalar.activation(kTt[:, ob, :], ps[:, :K5pad],
                                         AF.Identity, scale=0.125,
                                         bias=bk8x_t[:, ob:ob + 1])
                else:
                    nc.scalar.copy(v_sb[:, ob, :], ps[:])

            # distribute groups over geo iters (emit_group(j) at iter sched[j])
            sched = {}
            for j in range(NGRP):
                sched.setdefault(min(j * n_geo // NGRP, n_geo - 1), []).append(j)

            gathered = [False] * GBLK

            def _gather(blk):
                gathered[blk] = True
                gcnt = min(8, n_geo - blk * 8)
                nc.scalar.dma_start(
                    wgdT[0:gcnt * GM, blk, :, :],
                    wgd_dram[blk * 8:blk * 8 + gcnt]
                        .rearrange("g t h q -> (g t) h q"))

            def stage_b(g):
                wgdB = work4.tile([P, Q5pad], BF16, tag="wgdB")
                nc.vector.tensor_scalar(wgdB[:], wgps[g][:, :Q5pad],
                                        bgm1_t[:, g:g + 1],
                                        epsm1_t[:, g:g + 1],
                                        ALU.add, ALU.max)
                wgdm1 = work4.tile([P, Q5pad], BF16, tag="wgdm1")
                nc.gpsimd.tensor_tensor(wgdm1[:], wgdB[:], objqbc[:],
                                        ALU.mult)
                wgdM = work4.tile([P, Q5pad], BF16, tag="wgdM")
                nc.gpsimd.tensor_scalar(wgdM[:], wgdm1[:], 1.0, None, ALU.add)
                nc.scalar.dma_start(
                    wgd_dram[g].rearrange("t h q -> h t q"), wgdM[:])

            prev_groups = []
            for g in range(n_geo):
                blk = g // 8
                off = 64 * ((g % 8) // 4)
                q4 = g % 4
                mbase = g * GM
                lhs_wh = work4.tile([64, P], BF16, tag="lhs_wh")
                nc.vector.tensor_tensor(
                    lhs_wh[:].rearrange("k (h m) -> k h m", h=H),
                    w1e_f.rearrange("k (h m) -> k h m", h=H),
                    bankM[:, mbase:mbase + GM][:, None, :]
                        .to_broadcast((64, H, GM)),
                    ALU.mult)
                ups = psum_u.tile([P, 2, N], F32, tag="ups")
                for ci in range(2):
                    nc.tensor.matmul(ups[:, ci, :Q5pad],
                                     selap(off, off + 64, q4),
                                     dxy2[off:off + 64, blk, ci, :],
                                     start=True, stop=True)
                # projections fill the PE gap while DVE folds
                for j in sched.get(g, []):
                    emit_group(j)
                upsv = ups[:, :, :Q5pad]
                rrS = work4.tile([P, 2, Q5pad], F32, tag="rrS")
                nc.vector.tensor_scalar(rrS[:], upsv, MAGIC, -MAGIC,
                                        ALU.add, ALU.add)
                nfS = work4.tile([P, 2, Q5pad], F32, tag="nfS")
                nc.vector.tensor_tensor(nfS[:], rrS[:], upsv, ALU.subtract)
                # cos fold from nfS: nfC = nfS + (nfS <= -0.25)
                ind = work4.tile([P, 2, Q5pad], F32, tag="ind")
                nc.vector.tensor_scalar(ind[:], nfS[:], -0.25, None,
                                        ALU.is_le)
                nfC = work4.tile([P, 2, Q5pad], F32, tag="nfC")
                nc.vector.tensor_tensor(nfC[:], nfS[:], ind[:], ALU.add)
                sS = work4.tile([P, 2, Q5pad], BF16, tag="sS")
                nc.scalar.activation(sS[:], nfS[:], AF.Sin, scale=-TWO_PI)
                sC = work4.tile([P, 2, Q5pad], BF16, tag="sC")
                nc.scalar.activation(sC[:], nfC[:], AF.Sin, scale=-TWO_PI,
                                     bias=halfpi_t[:])
                wgp = psum_wg.tile([P, N], F32, tag="wgp")
                nc.tensor.matmul(wgp[:, :Q5pad], wblk(0), sS[:, 0, :],
                                 start=True, stop=False)
                nc.tensor.matmul(wgp[:, :Q5pad], wblk(1), sC[:, 0, :],
                                 start=False, stop=False)
                nc.tensor.matmul(wgp[:, :Q5pad], wblk(2), sS[:, 1, :],
                                 start=False, stop=False)
                nc.tensor.matmul(wgp[:, :Q5pad], wblk(3), sC[:, 1, :],
                                 start=False, stop=False)
                nc.tensor.matmul(wgp[:, :Q5pad], lhs_wh[:], bankN[:],
                                 start=False, stop=True)
                wgps[g] = wgp
                if g >= 1:
                    stage_b(g - 1)
                    if g - 1 == blk * 8 + 7:  # block complete -> gather early
                        _gather(blk)
                for j in prev_groups:
                    evict_group(j)
                prev_groups = sched.get(g, [])
            stage_b(n_geo - 1)
            for j in prev_groups:
                evict_group(j)
            for blk in range(GBLK):
                if not gathered[blk]:
                    _gather(blk)

        # ---------------- phase 5: attention (2-stage pipeline) -------------
        ot = persist.tile([P, NRB, N], BF16, tag="ot")
        with tc.tile_pool(name="work5", bufs=4) as work5, \
             tc.tile_pool(name="psum5", bufs=2, space="PSUM") as psum5, \
             tc.tile_pool(name="psum_s", bufs=1, space="PSUM") as psum_s, \
             tc.tile_pool(name="psum_r", bufs=1, space="PSUM") as psum_r, \
             tc.tile_pool(name="psum_av", bufs=1, space="PSUM") as psum_av:
            rb_order = list(range(GBLK, NRB5)) + list(range(GBLK))
            rpos = {rb: i for i, rb in enumerate(rb_order)}
            flat = [(ob, rb) for ob in range(NRB) for rb in rb_order]
            avs, sbanks, es = {}, {}, {}

            def accum(ob, rb):
                h0 = 2 * ob
                if rpos[rb] == 0:
                    av_t = psum_av.tile([P, N], F32, tag="avps")
                    sb_t = psum_s.tile([2, N], F32, tag="sbank")
                    avs[ob] = av_t
                    sbanks[ob] = sb_t
                e_ = es[(ob, rb)]
                first = rpos[rb] == 0
                last = rpos[rb] == NRB5 - 1
                for hi in range(2):
                    po = hi * DK
                    nc.tensor.matmul(sbanks[ob][:], oh2[:, 2 * hi:2 * hi + 2],
                                     e_[:, hi, :],
                                     start=(first and hi == 0),
                                     stop=(last and hi == 1),
                                     skip_group_check=True)
                    nc.tensor.matmul(avs[ob][po:po + DK, :],
                                     v_sb[:, rb,
                                          (h0 + hi) * DK:(h0 + hi + 1) * DK],
                                     e_[:, hi, :], start=first,
                                     stop=last,
                                     skip_group_check=True)

            def finish(ob):
                rs = work5.tile([2, N], F32, tag="rs")
                nc.vector.reciprocal(rs[:], sbanks[ob][:])
                rrp = psum_r.tile([P, N], F32, tag="rrp")
                nc.tensor.matmul(rrp[:], bc8_t, rs[:], start=True, stop=True)
                rr_b = work5.tile([P, N], F32, tag="rr_b")
                nc.scalar.copy(rr_b[:], rrp[:])
                nc.vector.tensor_tensor(ot[:, ob, :], avs[ob][:], rr_b[:],
                                        ALU.mult)

            for i, (ob, rb) in enumerate(flat):
                h0 = 2 * ob
                st2 = psum5.tile([P, 2, N], F32, tag="stps")
                for hi in range(2):
                    po = hi * DK
                    nc.tensor.matmul(
                        st2[:, hi, :],
                        kTt[po:po + DK, ob, rb * P:(rb + 1) * P],
                        qT[po:po + DK, ob, :], start=True, stop=True)
                e_ = work5.tile([P, 2, N], BF16, tag="e_t")
                nc.scalar.activation(e_[:], st2[:], AF.Exp,
                                     bias=mcol_t[:, rb:rb + 1])
                if rb < GBLK:
                    rows = min(P, G5pad - rb * P)
                    nc.vector.tensor_tensor(
                        e_[0:rows, :, 0:Q5pad], e_[0:rows, :, 0:Q5pad],
                        wgdT[0:rows, rb, h0:h0 + 2, :], ALU.mult)
                es[(ob, rb)] = e_
                if i >= 1:
                    accum(*flat[i - 1])
                    if rpos[flat[i - 1][1]] == NRB5 - 1:
                        finish(flat[i - 1][0])
            accum(*flat[-1])
            finish(flat[-1][0])

        # ---------------- phase 6: output projection ----------------
        with tc.tile_pool(name="work6", bufs=2) as work6, \
             tc.tile_pool(name="psum6", bufs=2, space="PSUM") as psum6:
            fps = []

            def evict(rr):
                fo = work6.tile([P, D], F32, tag="fo")
                if rr % 2 == 0:
                    nc.scalar.copy(fo[:], fps[rr][:])
                else:
                    nc.vector.tensor_copy(fo[:], fps[rr][:])
                nc.sync.dma_start(out[rr * P:(rr + 1) * P, :], fo[:])

            for r in range(NRB):
                ps = psum6.tile([P, D], F32, tag="fps")
                for kt in range(NRB):
                    nc.tensor.matmul(ps[:], ot[:, kt, r * P:(r + 1) * P],
                                     wsl(3, kt, 0, D),
                                     start=(kt == 0), stop=False)
                nc.tensor.matmul(ps[:], ones33_bf[32:33, :], borow,
                                 start=False, stop=True)
                fps.append(ps)
                if r >= 1:
                    evict(r - 1)
            evict(NRB - 1)

    _split_multi_waits(nc)
    return nc


_NC_CACHE = {}


def kernel(**inputs):
    in_maps, sizes, inv_q = _host_prep(inputs)
    if _NC_CACHE.get("sizes") != sizes:
        _NC_CACHE["nc"] = build_nc(*sizes)
        _NC_CACHE["sizes"] = sizes
    nc = _NC_CACHE["nc"]
    res = run_bass_kernel_spmd(nc, in_maps, list(range(B)))
    out = np.stack([res.results[b]["out"][inv_q[b]] for b in range(B)], axis=0)
    return out.astype(np.float32)


if __name__ == "__main__":
    print("kernel module ok")
